# revision 25
# baseline (speedup 1.0000x reference)
"""Trainium2 Bass kernel for nn_NeuronCircuitDown (moe_routing).

Math (per token t):
  y[t, :]  = sum_n iw[t, n] * (x[t, :] @ Wn[n])          # projection, Wn: [D, R]
  then K=4 sequential Householder reflections with vectors gathered from a
  32-row table by process_indices:
  y <- y - 2 * v * (v . y)   (table rows are unit-norm up to 1e-7, so the
                              reference's re-normalization is a no-op at
                              fp32 precision and is skipped)

Distribution: data-parallel over the 16384 tokens, 2048 tokens per core on 8
cores; weights/table replicated (weights pre-cast to fp16 / laid out on the
host — parameter prep only; all per-token compute happens on device).

Per-core device pipeline (tokens on partitions, 16 groups of 128):
  1. x loaded fp32 (Sync HWDGE), transposed per 128x128 block on the PE
     (fp32r transpose mode), evacuated PSUM->SBUF with an fp16 cast on
     ScalarE.  PE alternates transposes and matmuls so it never idles
     (keeps the HAM clock-gate warm); no DMA-xbar transposes (each one
     acts as a global DMA barrier on this hardware).
  2. fp16 matmuls: proj[t, r*8+n] accumulated over 8 K-chunks into PSUM
     (r-major layout so the n-reduction is contiguous)
  3. weighted sum on DVE: broadcast multiply (stride-0 AP) + contiguous
     reduction over n -> y fp32
  4. Householder: vectors pre-gathered via indirect DMA from a host-built
     pair table (rows = sqrt(2)*[P[j0] | P[j1]], indexed by j0*32+j1);
     batched multiply/reduce/multiply/subtract on DVE, run in two shard
     halves so the first half overlaps the second half's projection.
"""

import sys

if "/opt/trn_rl_repo" not in sys.path:
    sys.path.insert(0, "/opt/trn_rl_repo")

import numpy as np

B, S, D, R, N_IN, N_PROC, K = 4, 4096, 1024, 128, 8, 32, 4
N_CORES = 8
T_TOTAL = B * S
T = T_TOTAL // N_CORES   # tokens per core
G = T // 128             # token groups of 128 per core
KC = D // 128            # contraction chunks
HH_CHUNKS = [(0, 8), (8, 6), (14, 2)]  # (start, len) Householder chunks

_cache = {}
last_results = None


def _build_program():
    import concourse.bass as bass
    import concourse.mybir as mybir
    import concourse.tile as tile
    from concourse import bacc

    f32 = mybir.dt.float32
    f32r = mybir.dt.float32r
    f16 = mybir.dt.float16
    i32 = mybir.dt.int32
    mult = mybir.AluOpType.mult
    add = mybir.AluOpType.add
    sub = mybir.AluOpType.subtract
    X = mybir.AxisListType.X

    nc = bacc.Bacc(
        "TRN2",
        target_bir_lowering=False,
        debug=False,
        enable_asserts=False,
        num_devices=N_CORES,
    )

    x_d = nc.dram_tensor("x", [T, D], f32r, kind="ExternalInput").ap()
    iw_d = nc.dram_tensor("iw", [T, N_IN], f32, kind="ExternalInput").ap()
    idx_d = nc.dram_tensor("pidx", [T, K], i32, kind="ExternalInput").ap()
    wf_d = nc.dram_tensor("wflat", [D, R * N_IN], f16, kind="ExternalInput").ap()
    id_d = nc.dram_tensor("ident", [128, 128], f32r, kind="ExternalInput").ap()
    pp_d = nc.dram_tensor(
        "ppair", [N_PROC * N_PROC, 2 * R], f32, kind="ExternalInput"
    ).ap()
    out_d = nc.dram_tensor("out", [T, R], f32, kind="ExternalOutput").ap()

    x_re = x_d.rearrange("(g p) d -> p g d", p=128)       # [128, G, D]
    out_re = out_d.rearrange("(g p) r -> p g r", p=128)   # [128, G, R]

    with tile.TileContext(nc) as tc:
        with (
            tc.tile_pool(name="const", bufs=1) as cpool,
            tc.tile_pool(name="big", bufs=1) as bigpool,
            tc.tile_pool(name="xin", bufs=4) as xpool,
            tc.tile_pool(name="xt", bufs=4) as xtpool,
            tc.tile_pool(name="scl", bufs=3) as sclpool,
            tc.tile_pool(name="psxt", bufs=2, space="PSUM") as psA,
            tc.tile_pool(name="psproj", bufs=2, space="PSUM") as psB,
        ):
            # ---- constants / prefetches (small ones first; wf last so the
            # identity/indices don't queue behind its 2MB transfer) ----
            idm = cpool.tile([128, 128], f32r)
            nc.scalar.dma_start(idm[:], id_d[:])
            iw_sb = cpool.tile([128, G, N_IN], f32)
            nc.scalar.dma_start(iw_sb[:], iw_d.rearrange("(g p) n -> p g n", p=128))
            idx_sb = cpool.tile([128, G, K], i32)
            nc.scalar.dma_start(idx_sb[:], idx_d.rearrange("(g p) k -> p g k", p=128))
            wf_sb = cpool.tile([128, KC, R * N_IN], f16)
            nc.scalar.dma_start(wf_sb[:], wf_d.rearrange("(c p) m -> p c m", p=128))

            # pair indices j01 = 32*k0 + k1, j23 = 32*k2 + k3
            idx01 = cpool.tile([128, G], i32)
            idx23 = cpool.tile([128, G], i32)
            nc.vector.tensor_scalar(
                out=idx01[:], in0=idx_sb[:, :, 0], scalar1=N_PROC, scalar2=None,
                op0=mult,
            )
            nc.vector.tensor_tensor(
                out=idx01[:], in0=idx01[:], in1=idx_sb[:, :, 1], op=add
            )
            nc.vector.tensor_scalar(
                out=idx23[:], in0=idx_sb[:, :, 2], scalar1=N_PROC, scalar2=None,
                op0=mult,
            )
            nc.vector.tensor_tensor(
                out=idx23[:], in0=idx23[:], in1=idx_sb[:, :, 3], op=add
            )

            # gathered reflection vector pairs (already scaled by sqrt(2))
            v01 = bigpool.tile([128, G, 2 * R], f32)
            v23 = bigpool.tile([128, G, 2 * R], f32)
            for g in range(G):
                nc.gpsimd.indirect_dma_start(
                    out=v01[:, g, :],
                    out_offset=None,
                    in_=pp_d[:],
                    in_offset=bass.IndirectOffsetOnAxis(ap=idx01[:, g:g + 1], axis=0),
                )
                nc.gpsimd.indirect_dma_start(
                    out=v23[:, g, :],
                    out_offset=None,
                    in_=pp_d[:],
                    in_offset=bass.IndirectOffsetOnAxis(ap=idx23[:, g:g + 1], axis=0),
                )

            y_all = bigpool.tile([128, G, R], f32)
            hh_scr = bigpool.tile([128, G, R], f32)
            md = bigpool.tile([128, G], f32)
            v_views = [
                v01[:, :, 0:R],
                v01[:, :, R:2 * R],
                v23[:, :, 0:R],
                v23[:, :, R:2 * R],
            ]

            def hh_chunk(start, ln):
                s = slice(start, start + ln)
                for k in range(K):
                    vk = v_views[k][:, s, :]
                    nc.vector.tensor_tensor(
                        out=hh_scr[:, s, :], in0=y_all[:, s, :], in1=vk, op=mult
                    )
                    nc.vector.tensor_reduce(
                        out=md[:, s], in_=hh_scr[:, s, :], axis=X, op=add
                    )
                    nc.vector.tensor_tensor(
                        out=hh_scr[:, s, :],
                        in0=vk,
                        in1=md[:, s].unsqueeze(2).broadcast_to([128, ln, R]),
                        op=mult,
                    )
                    nc.vector.tensor_tensor(
                        out=y_all[:, s, :],
                        in0=y_all[:, s, :],
                        in1=hh_scr[:, s, :],
                        op=sub,
                    )
                nc.sync.dma_start(out_re[:, s, :], y_all[:, s, :])

            # ---- projection + weighted sum ----
            for g in range(G):
                x_g = xpool.tile([128, D], f32r, tag="x")
                nc.sync.dma_start(x_g[:], x_re[:, g, :])

                ps_xt = psA.tile([128, KC, 128], f32r, tag="psxt")
                for c in range(KC):
                    nc.tensor.transpose(
                        ps_xt[:, c, :], x_g[:, c * 128:(c + 1) * 128], idm[:]
                    )
                xt_g = xtpool.tile([128, KC, 128], f16, tag="xt")
                nc.scalar.copy(xt_g[:], ps_xt[:])

                ps_proj = psB.tile([128, R, N_IN], f32, tag="psproj")
                for c in range(KC):
                    for h in range(2):
                        nc.tensor.matmul(
                            ps_proj[:, 64 * h:64 * h + 64, :],
                            lhsT=xt_g[:, c, :],
                            rhs=wf_sb[:, c, h * 512:(h + 1) * 512],
                            start=(c == 0),
                            stop=(c == KC - 1),
                        )

                scaled = sclpool.tile([128, R, N_IN], f32, tag="scl")
                nc.vector.tensor_tensor(
                    out=scaled[:],
                    in0=ps_proj[:],
                    in1=iw_sb[:, g].unsqueeze(1).broadcast_to([128, R, N_IN]),
                    op=mult,
                )
                nc.vector.tensor_reduce(
                    out=y_all[:, g, :], in_=scaled[:], axis=X, op=add
                )

                for cs, cl in HH_CHUNKS[:-1]:
                    if g == cs + cl - 1:
                        hh_chunk(cs, cl)

            hh_chunk(*HH_CHUNKS[-1])

    nc.compile()
    return nc


def _get_program():
    if "nc" not in _cache:
        _cache["nc"] = _build_program()
    return _cache["nc"]


def _host_prep(x, input_weights, process_indices, input_neurons, process_neurons):
    xf = np.ascontiguousarray(np.asarray(x, dtype=np.float32)).reshape(T_TOTAL, D)
    iwf = np.ascontiguousarray(np.asarray(input_weights, dtype=np.float32)).reshape(
        T_TOTAL, N_IN
    )
    idxf = np.ascontiguousarray(np.asarray(process_indices, dtype=np.int32)).reshape(
        T_TOTAL, K
    )
    # W layout: wflat[d, r*8+n] = input_neurons[n, d, r]  (r-major, n innermost)
    wflat = np.ascontiguousarray(
        np.transpose(np.asarray(input_neurons, dtype=np.float32), (1, 2, 0)).reshape(
            D, R * N_IN
        )
    ).astype(np.float16)
    ident = np.eye(128, dtype=np.float32)
    # pair table: row j0*32+j1 = sqrt(2) * [P[j0] | P[j1]]
    # (sqrt(2) scaling turns y - (v*sqrt2)((v*sqrt2).y) into y - 2 v (v.y))
    pn = np.asarray(process_neurons, dtype=np.float32) * np.float32(np.sqrt(2.0))
    ppair = np.concatenate(
        [
            np.repeat(pn, N_PROC, axis=0),
            np.tile(pn, (N_PROC, 1)),
        ],
        axis=1,
    ).astype(np.float32)
    in_maps = []
    for c in range(N_CORES):
        sl = slice(c * T, (c + 1) * T)
        in_maps.append(
            {
                "x": xf[sl],
                "iw": iwf[sl],
                "pidx": idxf[sl],
                "wflat": wflat,
                "ident": ident,
                "ppair": ppair,
            }
        )
    return in_maps


def kernel(x, input_weights, process_indices, input_neurons, process_neurons):
    global last_results
    from concourse.bass_utils import run_bass_kernel_spmd

    nc = _get_program()
    in_maps = _host_prep(
        x, input_weights, process_indices, input_neurons, process_neurons
    )
    res = run_bass_kernel_spmd(nc, in_maps, core_ids=list(range(N_CORES)))
    last_results = res
    out = np.concatenate([res.results[c]["out"] for c in range(N_CORES)], axis=0)
    return out.reshape(B, S, R)


# revision 26
# speedup vs baseline: 1.0473x; 1.0473x over previous
"""Trainium2 Bass kernel for nn_NeuronCircuitDown (moe_routing).

Math (per token t):
  y[t, :]  = sum_n iw[t, n] * (x[t, :] @ Wn[n])          # projection, Wn: [D, R]
  then K=4 sequential Householder reflections with vectors gathered from a
  32-row table by process_indices:
  y <- y - 2 * v * (v . y)   (table rows are unit-norm up to 1e-7, so the
                              reference's re-normalization is a no-op at
                              fp32 precision and is skipped)

Distribution: data-parallel over the 16384 tokens, 2048 tokens per core on 8
cores; weights/table replicated (weights pre-cast to fp16 / laid out on the
host — parameter prep only; all per-token compute happens on device).

Per-core device pipeline (tokens on partitions, 16 groups of 128):
  1. x loaded fp32 (Sync HWDGE), transposed per 128x128 block on the PE
     (fp32r transpose mode), evacuated PSUM->SBUF with an fp16 cast on
     ScalarE.  PE alternates transposes and matmuls so it never idles
     (keeps the HAM clock-gate warm); no DMA-xbar transposes (each one
     acts as a global DMA barrier on this hardware).
  2. fp16 matmuls: proj[t, r*8+n] accumulated over 8 K-chunks into PSUM
     (r-major layout so the n-reduction is contiguous)
  3. weighted sum on DVE: broadcast multiply (stride-0 AP) + contiguous
     reduction over n -> y fp32
  4. Householder: vectors pre-gathered via indirect DMA from a host-built
     pair table (rows = sqrt(2)*[P[j0] | P[j1]], indexed by j0*32+j1);
     batched multiply/reduce/multiply/subtract on DVE, run in two shard
     halves so the first half overlaps the second half's projection.
"""

import sys

if "/opt/trn_rl_repo" not in sys.path:
    sys.path.insert(0, "/opt/trn_rl_repo")

import numpy as np

B, S, D, R, N_IN, N_PROC, K = 4, 4096, 1024, 128, 8, 32, 4
N_CORES = 8
T_TOTAL = B * S
T = T_TOTAL // N_CORES   # tokens per core
G = T // 128             # token groups of 128 per core
KC = D // 128            # contraction chunks
HH_CHUNKS = [(2 * i, 2) for i in range(8)]  # (start, len) Householder chunks

_cache = {}
last_results = None


def _build_program():
    import concourse.bass as bass
    import concourse.mybir as mybir
    import concourse.tile as tile
    from concourse import bacc

    f32 = mybir.dt.float32
    f32r = mybir.dt.float32r
    f16 = mybir.dt.float16
    i32 = mybir.dt.int32
    mult = mybir.AluOpType.mult
    add = mybir.AluOpType.add
    sub = mybir.AluOpType.subtract
    X = mybir.AxisListType.X

    nc = bacc.Bacc(
        "TRN2",
        target_bir_lowering=False,
        debug=False,
        enable_asserts=False,
        num_devices=N_CORES,
    )

    x_d = nc.dram_tensor("x", [T, D], f32r, kind="ExternalInput").ap()
    iw_d = nc.dram_tensor("iw", [T, N_IN], f32, kind="ExternalInput").ap()
    idx_d = nc.dram_tensor("pidx", [T, K], i32, kind="ExternalInput").ap()
    wf_d = nc.dram_tensor("wflat", [D, R * N_IN], f16, kind="ExternalInput").ap()
    id_d = nc.dram_tensor("ident", [128, 128], f32r, kind="ExternalInput").ap()
    pp_d = nc.dram_tensor(
        "ppair", [N_PROC * N_PROC, 2 * R], f32, kind="ExternalInput"
    ).ap()
    out_d = nc.dram_tensor("out", [T, R], f32, kind="ExternalOutput").ap()

    x_re = x_d.rearrange("(g p) d -> p g d", p=128)       # [128, G, D]
    out_re = out_d.rearrange("(g p) r -> p g r", p=128)   # [128, G, R]

    with tile.TileContext(nc) as tc:
        with (
            tc.tile_pool(name="const", bufs=1) as cpool,
            tc.tile_pool(name="big", bufs=1) as bigpool,
            tc.tile_pool(name="xin", bufs=4) as xpool,
            tc.tile_pool(name="xt", bufs=4) as xtpool,
            tc.tile_pool(name="scl", bufs=3) as sclpool,
            tc.tile_pool(name="psxt", bufs=2, space="PSUM") as psA,
            tc.tile_pool(name="psproj", bufs=2, space="PSUM") as psB,
        ):
            # ---- constants / prefetches (small ones first; wf last so the
            # identity/indices don't queue behind its 2MB transfer) ----
            idm = cpool.tile([128, 128], f32r)
            nc.scalar.dma_start(idm[:], id_d[:])
            iw_sb = cpool.tile([128, G, N_IN], f32)
            nc.scalar.dma_start(iw_sb[:], iw_d.rearrange("(g p) n -> p g n", p=128))
            idx_sb = cpool.tile([128, G, K], i32)
            nc.scalar.dma_start(idx_sb[:], idx_d.rearrange("(g p) k -> p g k", p=128))
            wf_sb = cpool.tile([128, KC, R * N_IN], f16)
            wf_re = wf_d.rearrange("(c p) m -> p c m", p=128)
            for c in range(KC):
                nc.scalar.dma_start(wf_sb[:, c:c + 1, :], wf_re[:, c:c + 1, :])

            # pair indices j01 = 32*k0 + k1, j23 = 32*k2 + k3
            idx01 = cpool.tile([128, G], i32)
            idx23 = cpool.tile([128, G], i32)
            nc.vector.tensor_scalar(
                out=idx01[:], in0=idx_sb[:, :, 0], scalar1=N_PROC, scalar2=None,
                op0=mult,
            )
            nc.vector.tensor_tensor(
                out=idx01[:], in0=idx01[:], in1=idx_sb[:, :, 1], op=add
            )
            nc.vector.tensor_scalar(
                out=idx23[:], in0=idx_sb[:, :, 2], scalar1=N_PROC, scalar2=None,
                op0=mult,
            )
            nc.vector.tensor_tensor(
                out=idx23[:], in0=idx23[:], in1=idx_sb[:, :, 3], op=add
            )

            # gathered reflection vector pairs (already scaled by sqrt(2))
            v01 = bigpool.tile([128, G, 2 * R], f32)
            v23 = bigpool.tile([128, G, 2 * R], f32)
            for g in range(G):
                nc.gpsimd.indirect_dma_start(
                    out=v01[:, g, :],
                    out_offset=None,
                    in_=pp_d[:],
                    in_offset=bass.IndirectOffsetOnAxis(ap=idx01[:, g:g + 1], axis=0),
                )
                nc.gpsimd.indirect_dma_start(
                    out=v23[:, g, :],
                    out_offset=None,
                    in_=pp_d[:],
                    in_offset=bass.IndirectOffsetOnAxis(ap=idx23[:, g:g + 1], axis=0),
                )

            y_all = bigpool.tile([128, G, R], f32)
            hh_scr = bigpool.tile([128, G, R], f32)
            md = bigpool.tile([128, G], f32)
            v_views = [
                v01[:, :, 0:R],
                v01[:, :, R:2 * R],
                v23[:, :, 0:R],
                v23[:, :, R:2 * R],
            ]

            def hh_chunk(start, ln):
                s = slice(start, start + ln)
                for k in range(K):
                    vk = v_views[k][:, s, :]
                    nc.vector.tensor_tensor(
                        out=hh_scr[:, s, :], in0=y_all[:, s, :], in1=vk, op=mult
                    )
                    nc.vector.tensor_reduce(
                        out=md[:, s], in_=hh_scr[:, s, :], axis=X, op=add
                    )
                    nc.vector.tensor_tensor(
                        out=hh_scr[:, s, :],
                        in0=vk,
                        in1=md[:, s].unsqueeze(2).broadcast_to([128, ln, R]),
                        op=mult,
                    )
                    nc.vector.tensor_tensor(
                        out=y_all[:, s, :],
                        in0=y_all[:, s, :],
                        in1=hh_scr[:, s, :],
                        op=sub,
                    )
                nc.sync.dma_start(out_re[:, s, :], y_all[:, s, :])

            # ---- projection + weighted sum ----
            for g in range(G):
                x_g = xpool.tile([128, D], f32r, tag="x")
                nc.sync.dma_start(x_g[:], x_re[:, g, :])

                ps_xt = psA.tile([128, KC, 128], f32r, tag="psxt")
                for c in range(KC):
                    nc.tensor.transpose(
                        ps_xt[:, c, :], x_g[:, c * 128:(c + 1) * 128], idm[:]
                    )
                xt_g = xtpool.tile([128, KC, 128], f16, tag="xt")
                nc.scalar.copy(xt_g[:], ps_xt[:])

                ps_proj = psB.tile([128, R, N_IN], f32, tag="psproj")
                for c in range(KC):
                    for h in range(2):
                        nc.tensor.matmul(
                            ps_proj[:, 64 * h:64 * h + 64, :],
                            lhsT=xt_g[:, c, :],
                            rhs=wf_sb[:, c, h * 512:(h + 1) * 512],
                            start=(c == 0),
                            stop=(c == KC - 1),
                        )

                scaled = sclpool.tile([128, R, N_IN], f32, tag="scl")
                nc.vector.tensor_tensor(
                    out=scaled[:],
                    in0=ps_proj[:],
                    in1=iw_sb[:, g].unsqueeze(1).broadcast_to([128, R, N_IN]),
                    op=mult,
                )
                nc.vector.tensor_reduce(
                    out=y_all[:, g, :], in_=scaled[:], axis=X, op=add
                )

                for cs, cl in HH_CHUNKS[:-1]:
                    if g == cs + cl - 1:
                        hh_chunk(cs, cl)

            hh_chunk(*HH_CHUNKS[-1])

    nc.compile()
    return nc


def _get_program():
    if "nc" not in _cache:
        _cache["nc"] = _build_program()
    return _cache["nc"]


def _host_prep(x, input_weights, process_indices, input_neurons, process_neurons):
    xf = np.ascontiguousarray(np.asarray(x, dtype=np.float32)).reshape(T_TOTAL, D)
    iwf = np.ascontiguousarray(np.asarray(input_weights, dtype=np.float32)).reshape(
        T_TOTAL, N_IN
    )
    idxf = np.ascontiguousarray(np.asarray(process_indices, dtype=np.int32)).reshape(
        T_TOTAL, K
    )
    # W layout: wflat[d, r*8+n] = input_neurons[n, d, r]  (r-major, n innermost)
    wflat = np.ascontiguousarray(
        np.transpose(np.asarray(input_neurons, dtype=np.float32), (1, 2, 0)).reshape(
            D, R * N_IN
        )
    ).astype(np.float16)
    ident = np.eye(128, dtype=np.float32)
    # pair table: row j0*32+j1 = sqrt(2) * [P[j0] | P[j1]]
    # (sqrt(2) scaling turns y - (v*sqrt2)((v*sqrt2).y) into y - 2 v (v.y))
    pn = np.asarray(process_neurons, dtype=np.float32) * np.float32(np.sqrt(2.0))
    ppair = np.concatenate(
        [
            np.repeat(pn, N_PROC, axis=0),
            np.tile(pn, (N_PROC, 1)),
        ],
        axis=1,
    ).astype(np.float32)
    in_maps = []
    for c in range(N_CORES):
        sl = slice(c * T, (c + 1) * T)
        in_maps.append(
            {
                "x": xf[sl],
                "iw": iwf[sl],
                "pidx": idxf[sl],
                "wflat": wflat,
                "ident": ident,
                "ppair": ppair,
            }
        )
    return in_maps


def kernel(x, input_weights, process_indices, input_neurons, process_neurons):
    global last_results
    from concourse.bass_utils import run_bass_kernel_spmd

    nc = _get_program()
    in_maps = _host_prep(
        x, input_weights, process_indices, input_neurons, process_neurons
    )
    res = run_bass_kernel_spmd(nc, in_maps, core_ids=list(range(N_CORES)))
    last_results = res
    out = np.concatenate([res.results[c]["out"] for c in range(N_CORES)], axis=0)
    return out.reshape(B, S, R)


# revision 27
# speedup vs baseline: 1.0765x; 1.0279x over previous
"""Trainium2 Bass kernel for nn_NeuronCircuitDown (moe_routing).

Math (per token t):
  y[t, :]  = sum_n iw[t, n] * (x[t, :] @ Wn[n])          # projection, Wn: [D, R]
  then K=4 sequential Householder reflections with vectors gathered from a
  32-row table by process_indices:
  y <- y - 2 * v * (v . y)   (table rows are unit-norm up to 1e-7, so the
                              reference's re-normalization is a no-op at
                              fp32 precision and is skipped)

Distribution: data-parallel over the 16384 tokens, 2048 tokens per core on 8
cores; weights/table replicated (weights pre-cast to fp16 / laid out on the
host — parameter prep only; all per-token compute happens on device).

Per-core device pipeline (tokens on partitions, 16 groups of 128):
  1. x loaded fp32 (Sync HWDGE), transposed per 128x128 block on the PE
     (fp32r transpose mode), evacuated PSUM->SBUF with an fp16 cast on
     ScalarE.  PE alternates transposes and matmuls so it never idles
     (keeps the HAM clock-gate warm); no DMA-xbar transposes (each one
     acts as a global DMA barrier on this hardware).
  2. fp16 matmuls: proj[t, r*8+n] accumulated over 8 K-chunks into PSUM
     (r-major layout so the n-reduction is contiguous)
  3. weighted sum on DVE: broadcast multiply (stride-0 AP) + contiguous
     reduction over n -> y fp32
  4. Householder: vectors pre-gathered via indirect DMA from a host-built
     pair table (rows = sqrt(2)*[P[j0] | P[j1]], indexed by j0*32+j1);
     batched multiply/reduce/multiply/subtract on DVE, run in two shard
     halves so the first half overlaps the second half's projection.
"""

import sys

if "/opt/trn_rl_repo" not in sys.path:
    sys.path.insert(0, "/opt/trn_rl_repo")

import numpy as np

B, S, D, R, N_IN, N_PROC, K = 4, 4096, 1024, 128, 8, 32, 4
N_CORES = 8
T_TOTAL = B * S
T = T_TOTAL // N_CORES   # tokens per core
G = T // 128             # token groups of 128 per core
KC = D // 128            # contraction chunks
HH_CHUNKS = [(4 * i, 4) for i in range(4)]  # (start, len) Householder chunks

_cache = {}
last_results = None


def _build_program():
    import concourse.bass as bass
    import concourse.mybir as mybir
    import concourse.tile as tile
    from concourse import bacc

    f32 = mybir.dt.float32
    f32r = mybir.dt.float32r
    f16 = mybir.dt.float16
    i32 = mybir.dt.int32
    mult = mybir.AluOpType.mult
    add = mybir.AluOpType.add
    sub = mybir.AluOpType.subtract
    X = mybir.AxisListType.X

    nc = bacc.Bacc(
        "TRN2",
        target_bir_lowering=False,
        debug=False,
        enable_asserts=False,
        num_devices=N_CORES,
    )

    x_d = nc.dram_tensor("x", [T, D], f32r, kind="ExternalInput").ap()
    iw_d = nc.dram_tensor("iw", [T, N_IN], f32, kind="ExternalInput").ap()
    idx_d = nc.dram_tensor("pidx", [T, K], i32, kind="ExternalInput").ap()
    wf_d = nc.dram_tensor("wflat", [D, R * N_IN], f16, kind="ExternalInput").ap()
    id_d = nc.dram_tensor("ident", [128, 128], f32r, kind="ExternalInput").ap()
    pp_d = nc.dram_tensor(
        "ppair", [N_PROC * N_PROC, 2 * R], f32, kind="ExternalInput"
    ).ap()
    out_d = nc.dram_tensor("out", [T, R], f32, kind="ExternalOutput").ap()

    x_re = x_d.rearrange("(g p) d -> p g d", p=128)       # [128, G, D]
    out_re = out_d.rearrange("(g p) r -> p g r", p=128)   # [128, G, R]

    with tile.TileContext(nc) as tc:
        with (
            tc.tile_pool(name="const", bufs=1) as cpool,
            tc.tile_pool(name="big", bufs=1) as bigpool,
            tc.tile_pool(name="xin", bufs=4) as xpool,
            tc.tile_pool(name="xt", bufs=4) as xtpool,
            tc.tile_pool(name="scl", bufs=4) as sclpool,
            tc.tile_pool(name="psxt", bufs=2, space="PSUM") as psA,
            tc.tile_pool(name="psproj", bufs=2, space="PSUM") as psB,
        ):
            # ---- constants / prefetches (small ones first; wf last so the
            # identity/indices don't queue behind its 2MB transfer) ----
            idm = cpool.tile([128, 128], f32r)
            nc.scalar.dma_start(idm[:], id_d[:])
            wf_sb = cpool.tile([128, KC, R * N_IN], f16)
            wf_re = wf_d.rearrange("(c p) m -> p c m", p=128)
            for c in range(KC):
                nc.scalar.dma_start(wf_sb[:, c:c + 1, :], wf_re[:, c:c + 1, :])
            iw_sb = cpool.tile([128, G, N_IN], f32)
            nc.scalar.dma_start(iw_sb[:], iw_d.rearrange("(g p) n -> p g n", p=128))
            idx_sb = cpool.tile([128, G, K], i32)
            nc.scalar.dma_start(idx_sb[:], idx_d.rearrange("(g p) k -> p g k", p=128))

            # pair indices j01 = 32*k0 + k1, j23 = 32*k2 + k3
            idx01 = cpool.tile([128, G], i32)
            idx23 = cpool.tile([128, G], i32)
            nc.vector.tensor_scalar(
                out=idx01[:], in0=idx_sb[:, :, 0], scalar1=N_PROC, scalar2=None,
                op0=mult,
            )
            nc.vector.tensor_tensor(
                out=idx01[:], in0=idx01[:], in1=idx_sb[:, :, 1], op=add
            )
            nc.vector.tensor_scalar(
                out=idx23[:], in0=idx_sb[:, :, 2], scalar1=N_PROC, scalar2=None,
                op0=mult,
            )
            nc.vector.tensor_tensor(
                out=idx23[:], in0=idx23[:], in1=idx_sb[:, :, 3], op=add
            )

            # gathered reflection vector pairs (already scaled by sqrt(2))
            v01 = bigpool.tile([128, G, 2 * R], f32)
            v23 = bigpool.tile([128, G, 2 * R], f32)
            for g in range(G):
                nc.gpsimd.indirect_dma_start(
                    out=v01[:, g, :],
                    out_offset=None,
                    in_=pp_d[:],
                    in_offset=bass.IndirectOffsetOnAxis(ap=idx01[:, g:g + 1], axis=0),
                )
                nc.gpsimd.indirect_dma_start(
                    out=v23[:, g, :],
                    out_offset=None,
                    in_=pp_d[:],
                    in_offset=bass.IndirectOffsetOnAxis(ap=idx23[:, g:g + 1], axis=0),
                )

            y_all = bigpool.tile([128, G, R], f32)
            hh_scr = bigpool.tile([128, G, R], f32)
            md = bigpool.tile([128, G], f32)
            v_views = [
                v01[:, :, 0:R],
                v01[:, :, R:2 * R],
                v23[:, :, 0:R],
                v23[:, :, R:2 * R],
            ]

            def hh_chunk(start, ln):
                s = slice(start, start + ln)
                for k in range(K):
                    vk = v_views[k][:, s, :]
                    nc.vector.tensor_tensor(
                        out=hh_scr[:, s, :], in0=y_all[:, s, :], in1=vk, op=mult
                    )
                    nc.vector.tensor_reduce(
                        out=md[:, s], in_=hh_scr[:, s, :], axis=X, op=add
                    )
                    nc.vector.tensor_tensor(
                        out=hh_scr[:, s, :],
                        in0=vk,
                        in1=md[:, s].unsqueeze(2).broadcast_to([128, ln, R]),
                        op=mult,
                    )
                    nc.vector.tensor_tensor(
                        out=y_all[:, s, :],
                        in0=y_all[:, s, :],
                        in1=hh_scr[:, s, :],
                        op=sub,
                    )
                nc.sync.dma_start(out_re[:, s, :], y_all[:, s, :])

            # ---- projection + weighted sum ----
            for g in range(G):
                x_g = xpool.tile([128, D], f32r, tag="x")
                nc.sync.dma_start(x_g[:], x_re[:, g, :])

                ps_xt = psA.tile([128, KC, 128], f32r, tag="psxt")
                for c in range(KC):
                    nc.tensor.transpose(
                        ps_xt[:, c, :], x_g[:, c * 128:(c + 1) * 128], idm[:]
                    )
                xt_g = xtpool.tile([128, KC, 128], f16, tag="xt")
                nc.scalar.copy(xt_g[:], ps_xt[:])

                ps_proj = psB.tile([128, R, N_IN], f32, tag="psproj")
                for c in range(KC):
                    for h in range(2):
                        nc.tensor.matmul(
                            ps_proj[:, 64 * h:64 * h + 64, :],
                            lhsT=xt_g[:, c, :],
                            rhs=wf_sb[:, c, h * 512:(h + 1) * 512],
                            start=(c == 0),
                            stop=(c == KC - 1),
                        )

                scaled = sclpool.tile([128, R, N_IN], f32, tag="scl")
                nc.vector.tensor_tensor(
                    out=scaled[:],
                    in0=ps_proj[:],
                    in1=iw_sb[:, g].unsqueeze(1).broadcast_to([128, R, N_IN]),
                    op=mult,
                )
                nc.vector.tensor_reduce(
                    out=y_all[:, g, :], in_=scaled[:], axis=X, op=add
                )

                for cs, cl in HH_CHUNKS[:-1]:
                    if g == cs + cl - 1:
                        hh_chunk(cs, cl)

            hh_chunk(*HH_CHUNKS[-1])

    nc.compile()
    return nc


def _get_program():
    if "nc" not in _cache:
        _cache["nc"] = _build_program()
    return _cache["nc"]


def _host_prep(x, input_weights, process_indices, input_neurons, process_neurons):
    xf = np.ascontiguousarray(np.asarray(x, dtype=np.float32)).reshape(T_TOTAL, D)
    iwf = np.ascontiguousarray(np.asarray(input_weights, dtype=np.float32)).reshape(
        T_TOTAL, N_IN
    )
    idxf = np.ascontiguousarray(np.asarray(process_indices, dtype=np.int32)).reshape(
        T_TOTAL, K
    )
    # W layout: wflat[d, r*8+n] = input_neurons[n, d, r]  (r-major, n innermost)
    wflat = np.ascontiguousarray(
        np.transpose(np.asarray(input_neurons, dtype=np.float32), (1, 2, 0)).reshape(
            D, R * N_IN
        )
    ).astype(np.float16)
    ident = np.eye(128, dtype=np.float32)
    # pair table: row j0*32+j1 = sqrt(2) * [P[j0] | P[j1]]
    # (sqrt(2) scaling turns y - (v*sqrt2)((v*sqrt2).y) into y - 2 v (v.y))
    pn = np.asarray(process_neurons, dtype=np.float32) * np.float32(np.sqrt(2.0))
    ppair = np.concatenate(
        [
            np.repeat(pn, N_PROC, axis=0),
            np.tile(pn, (N_PROC, 1)),
        ],
        axis=1,
    ).astype(np.float32)
    in_maps = []
    for c in range(N_CORES):
        sl = slice(c * T, (c + 1) * T)
        in_maps.append(
            {
                "x": xf[sl],
                "iw": iwf[sl],
                "pidx": idxf[sl],
                "wflat": wflat,
                "ident": ident,
                "ppair": ppair,
            }
        )
    return in_maps


def kernel(x, input_weights, process_indices, input_neurons, process_neurons):
    global last_results
    from concourse.bass_utils import run_bass_kernel_spmd

    nc = _get_program()
    in_maps = _host_prep(
        x, input_weights, process_indices, input_neurons, process_neurons
    )
    res = run_bass_kernel_spmd(nc, in_maps, core_ids=list(range(N_CORES)))
    last_results = res
    out = np.concatenate([res.results[c]["out"] for c in range(N_CORES)], axis=0)
    return out.reshape(B, S, R)


# revision 28
# speedup vs baseline: 1.1190x; 1.0395x over previous
"""Trainium2 Bass kernel for nn_NeuronCircuitDown (moe_routing).

Math (per token t):
  y[t, :]  = sum_n iw[t, n] * (x[t, :] @ Wn[n])          # projection, Wn: [D, R]
  then K=4 sequential Householder reflections with vectors gathered from a
  32-row table by process_indices:
  y <- y - 2 * v * (v . y)   (table rows are unit-norm up to 1e-7, so the
                              reference's re-normalization is a no-op at
                              fp32 precision and is skipped)

Distribution: data-parallel over the 16384 tokens, 2048 tokens per core on 8
cores; weights/table replicated (weights pre-cast to fp16 / laid out on the
host — parameter prep only; all per-token compute happens on device).

Per-core device pipeline (tokens on partitions, 16 groups of 128):
  1. x loaded fp32 (Sync HWDGE), transposed per 128x128 block on the PE
     (fp32r transpose mode), evacuated PSUM->SBUF with an fp16 cast on
     ScalarE.  PE alternates transposes and matmuls so it never idles
     (keeps the HAM clock-gate warm); no DMA-xbar transposes (each one
     acts as a global DMA barrier on this hardware).
  2. fp16 matmuls: proj[t, r*8+n] accumulated over 8 K-chunks into PSUM
     (r-major layout so the n-reduction is contiguous)
  3. weighted sum on DVE: broadcast multiply (stride-0 AP) + contiguous
     reduction over n -> y fp32
  4. Householder: vectors pre-gathered via indirect DMA from a host-built
     pair table (rows = sqrt(2)*[P[j0] | P[j1]], indexed by j0*32+j1);
     batched multiply/reduce/multiply/subtract on DVE, run in two shard
     halves so the first half overlaps the second half's projection.
"""

import sys

if "/opt/trn_rl_repo" not in sys.path:
    sys.path.insert(0, "/opt/trn_rl_repo")

import numpy as np

B, S, D, R, N_IN, N_PROC, K = 4, 4096, 1024, 128, 8, 32, 4
N_CORES = 8
T_TOTAL = B * S
T = T_TOTAL // N_CORES   # tokens per core
G = T // 128             # token groups of 128 per core
KC = D // 128            # contraction chunks
HH_CHUNKS = [(4 * i, 4) for i in range(4)]  # (start, len) Householder chunks

_cache = {}
last_results = None


def _build_program():
    import concourse.bass as bass
    import concourse.mybir as mybir
    import concourse.tile as tile
    from concourse import bacc

    f32 = mybir.dt.float32
    f32r = mybir.dt.float32r
    f16 = mybir.dt.float16
    i32 = mybir.dt.int32
    mult = mybir.AluOpType.mult
    add = mybir.AluOpType.add
    sub = mybir.AluOpType.subtract
    X = mybir.AxisListType.X

    nc = bacc.Bacc(
        "TRN2",
        target_bir_lowering=False,
        debug=False,
        enable_asserts=False,
        num_devices=N_CORES,
    )

    x_d = nc.dram_tensor("x", [T, D], f32r, kind="ExternalInput").ap()
    iw_d = nc.dram_tensor("iw", [T, N_IN], f32, kind="ExternalInput").ap()
    idx_d = nc.dram_tensor("pidx", [T, K], i32, kind="ExternalInput").ap()
    wf_d = nc.dram_tensor("wflat", [D, R * N_IN], f16, kind="ExternalInput").ap()
    id_d = nc.dram_tensor("ident", [128, 128], f32r, kind="ExternalInput").ap()
    pp_d = nc.dram_tensor(
        "ppair", [N_PROC * N_PROC, 2 * R], f16, kind="ExternalInput"
    ).ap()
    out_d = nc.dram_tensor("out", [T, R], f32, kind="ExternalOutput").ap()

    x_re = x_d.rearrange("(g p) d -> p g d", p=128)       # [128, G, D]
    out_re = out_d.rearrange("(g p) r -> p g r", p=128)   # [128, G, R]

    with tile.TileContext(nc) as tc:
        with (
            tc.tile_pool(name="const", bufs=1) as cpool,
            tc.tile_pool(name="big", bufs=1) as bigpool,
            tc.tile_pool(name="xin", bufs=4) as xpool,
            tc.tile_pool(name="xt", bufs=4) as xtpool,
            tc.tile_pool(name="scl", bufs=4) as sclpool,
            tc.tile_pool(name="psxt", bufs=2, space="PSUM") as psA,
            tc.tile_pool(name="psproj", bufs=2, space="PSUM") as psB,
        ):
            # ---- constants / prefetches (small ones first; wf last so the
            # identity/indices don't queue behind its 2MB transfer) ----
            idm = cpool.tile([128, 128], f32r)
            nc.scalar.dma_start(idm[:], id_d[:])
            wf_sb = cpool.tile([128, KC, R * N_IN], f16)
            wf_re = wf_d.rearrange("(c p) m -> p c m", p=128)
            for c in range(KC):
                eng = nc.scalar if c % 2 == 0 else nc.sync
                eng.dma_start(wf_sb[:, c:c + 1, :], wf_re[:, c:c + 1, :])
            iw_sb = cpool.tile([128, G, N_IN], f32)
            nc.scalar.dma_start(iw_sb[:], iw_d.rearrange("(g p) n -> p g n", p=128))
            idx_sb = cpool.tile([128, G, K], i32)
            nc.scalar.dma_start(idx_sb[:], idx_d.rearrange("(g p) k -> p g k", p=128))

            # pair indices j01 = 32*k0 + k1, j23 = 32*k2 + k3
            idx01 = cpool.tile([128, G], i32)
            idx23 = cpool.tile([128, G], i32)
            nc.vector.tensor_scalar(
                out=idx01[:], in0=idx_sb[:, :, 0], scalar1=N_PROC, scalar2=None,
                op0=mult,
            )
            nc.vector.tensor_tensor(
                out=idx01[:], in0=idx01[:], in1=idx_sb[:, :, 1], op=add
            )
            nc.vector.tensor_scalar(
                out=idx23[:], in0=idx_sb[:, :, 2], scalar1=N_PROC, scalar2=None,
                op0=mult,
            )
            nc.vector.tensor_tensor(
                out=idx23[:], in0=idx23[:], in1=idx_sb[:, :, 3], op=add
            )

            # gathered reflection vector pairs (already scaled by sqrt(2))
            v01 = bigpool.tile([128, G, 2 * R], f16)
            v23 = bigpool.tile([128, G, 2 * R], f16)
            for g in range(G):
                nc.gpsimd.indirect_dma_start(
                    out=v01[:, g, :],
                    out_offset=None,
                    in_=pp_d[:],
                    in_offset=bass.IndirectOffsetOnAxis(ap=idx01[:, g:g + 1], axis=0),
                )
                nc.gpsimd.indirect_dma_start(
                    out=v23[:, g, :],
                    out_offset=None,
                    in_=pp_d[:],
                    in_offset=bass.IndirectOffsetOnAxis(ap=idx23[:, g:g + 1], axis=0),
                )

            y_all = bigpool.tile([128, G, R], f16)
            hh_scr = bigpool.tile([128, G, R], f16)
            md = bigpool.tile([128, G], f16)
            mdb = bigpool.tile([128, 4, R], f16)
            v_views = [
                v01[:, :, 0:R],
                v01[:, :, R:2 * R],
                v23[:, :, 0:R],
                v23[:, :, R:2 * R],
            ]

            def hh_chunk(start, ln):
                s = slice(start, start + ln)
                for k in range(K):
                    vk = v_views[k][:, s, :]
                    nc.vector.tensor_tensor(
                        out=hh_scr[:, s, :], in0=y_all[:, s, :], in1=vk, op=mult
                    )
                    with nc.allow_low_precision("fp16 Householder dots"):
                        nc.vector.tensor_reduce(
                            out=md[:, s], in_=hh_scr[:, s, :], axis=X, op=add
                        )
                    # materialize the broadcast on ScalarE so the DVE multiply
                    # keeps its unit-stride 2x mode
                    nc.scalar.copy(
                        mdb[:, 0:ln, :],
                        md[:, s].unsqueeze(2).broadcast_to([128, ln, R]),
                    )
                    nc.vector.tensor_tensor(
                        out=hh_scr[:, s, :], in0=vk, in1=mdb[:, 0:ln, :], op=mult
                    )
                    nc.vector.tensor_tensor(
                        out=y_all[:, s, :],
                        in0=y_all[:, s, :],
                        in1=hh_scr[:, s, :],
                        op=sub,
                    )
                y32 = sclpool.tile([128, ln, R], f32, tag="y32")
                nc.vector.tensor_copy(y32[:], y_all[:, s, :])
                nc.sync.dma_start(out_re[:, s, :], y32[:])

            # ---- projection + weighted sum ----
            for g in range(G):
                x_g = xpool.tile([128, D], f32r, tag="x")
                nc.sync.dma_start(x_g[:], x_re[:, g, :])

                ps_xt = psA.tile([128, KC, 128], f32r, tag="psxt")
                for c in range(KC):
                    nc.tensor.transpose(
                        ps_xt[:, c, :], x_g[:, c * 128:(c + 1) * 128], idm[:]
                    )
                xt_g = xtpool.tile([128, KC, 128], f16, tag="xt")
                nc.scalar.copy(xt_g[:], ps_xt[:])

                ps_proj = psB.tile([128, R, N_IN], f32, tag="psproj")
                for c in range(KC):
                    for h in range(2):
                        nc.tensor.matmul(
                            ps_proj[:, 64 * h:64 * h + 64, :],
                            lhsT=xt_g[:, c, :],
                            rhs=wf_sb[:, c, h * 512:(h + 1) * 512],
                            start=(c == 0),
                            stop=(c == KC - 1),
                        )

                scaled = sclpool.tile([128, R, N_IN], f32, tag="scl")
                nc.vector.tensor_tensor(
                    out=scaled[:],
                    in0=ps_proj[:],
                    in1=iw_sb[:, g].unsqueeze(1).broadcast_to([128, R, N_IN]),
                    op=mult,
                )
                with nc.allow_low_precision("fp16 y for Householder"):
                    nc.vector.tensor_reduce(
                        out=y_all[:, g, :], in_=scaled[:], axis=X, op=add
                    )

                for cs, cl in HH_CHUNKS[:-1]:
                    if g == cs + cl - 1:
                        hh_chunk(cs, cl)

            hh_chunk(*HH_CHUNKS[-1])

    nc.compile()
    return nc


def _get_program():
    if "nc" not in _cache:
        _cache["nc"] = _build_program()
    return _cache["nc"]


def _host_prep(x, input_weights, process_indices, input_neurons, process_neurons):
    xf = np.ascontiguousarray(np.asarray(x, dtype=np.float32)).reshape(T_TOTAL, D)
    iwf = np.ascontiguousarray(np.asarray(input_weights, dtype=np.float32)).reshape(
        T_TOTAL, N_IN
    )
    idxf = np.ascontiguousarray(np.asarray(process_indices, dtype=np.int32)).reshape(
        T_TOTAL, K
    )
    # W layout: wflat[d, r*8+n] = input_neurons[n, d, r]  (r-major, n innermost)
    wflat = np.ascontiguousarray(
        np.transpose(np.asarray(input_neurons, dtype=np.float32), (1, 2, 0)).reshape(
            D, R * N_IN
        )
    ).astype(np.float16)
    ident = np.eye(128, dtype=np.float32)
    # pair table: row j0*32+j1 = sqrt(2) * [P[j0] | P[j1]]
    # (sqrt(2) scaling turns y - (v*sqrt2)((v*sqrt2).y) into y - 2 v (v.y))
    pn = np.asarray(process_neurons, dtype=np.float32) * np.float32(np.sqrt(2.0))
    ppair = np.concatenate(
        [
            np.repeat(pn, N_PROC, axis=0),
            np.tile(pn, (N_PROC, 1)),
        ],
        axis=1,
    ).astype(np.float16)
    in_maps = []
    for c in range(N_CORES):
        sl = slice(c * T, (c + 1) * T)
        in_maps.append(
            {
                "x": xf[sl],
                "iw": iwf[sl],
                "pidx": idxf[sl],
                "wflat": wflat,
                "ident": ident,
                "ppair": ppair,
            }
        )
    return in_maps


def kernel(x, input_weights, process_indices, input_neurons, process_neurons):
    global last_results
    from concourse.bass_utils import run_bass_kernel_spmd

    nc = _get_program()
    in_maps = _host_prep(
        x, input_weights, process_indices, input_neurons, process_neurons
    )
    res = run_bass_kernel_spmd(nc, in_maps, core_ids=list(range(N_CORES)))
    last_results = res
    out = np.concatenate([res.results[c]["out"] for c in range(N_CORES)], axis=0)
    return out.reshape(B, S, R)


# revision 30
# speedup vs baseline: 1.1260x; 1.0062x over previous
"""Trainium2 Bass kernel for nn_NeuronCircuitDown (moe_routing).

Math (per token t):
  y[t, :]  = sum_n iw[t, n] * (x[t, :] @ Wn[n])          # projection, Wn: [D, R]
  then K=4 sequential Householder reflections with vectors gathered from a
  32-row table by process_indices:
  y <- y - 2 * v * (v . y)   (table rows are unit-norm up to 1e-7, so the
                              reference's re-normalization is a no-op at
                              fp32 precision and is skipped)

Distribution: data-parallel over the 16384 tokens, 2048 tokens per core on 8
cores; weights/table replicated (weights pre-cast to fp16 / laid out on the
host — parameter prep only; all per-token compute happens on device).

Per-core device pipeline (tokens on partitions, 16 groups of 128):
  1. x loaded fp32 (Sync HWDGE), transposed per 128x128 block on the PE
     (fp32r transpose mode), evacuated PSUM->SBUF with an fp16 cast on
     ScalarE.  PE alternates transposes and matmuls so it never idles
     (keeps the HAM clock-gate warm); no DMA-xbar transposes (each one
     acts as a global DMA barrier on this hardware).
  2. fp16 matmuls: proj[t, r*8+n] accumulated over 8 K-chunks into PSUM
     (r-major layout so the n-reduction is contiguous)
  3. weighted sum on DVE: broadcast multiply (stride-0 AP) + contiguous
     reduction over n -> y fp32
  4. Householder: vectors pre-gathered via indirect DMA from a host-built
     pair table (rows = sqrt(2)*[P[j0] | P[j1]], indexed by j0*32+j1);
     batched multiply/reduce/multiply/subtract on DVE, run in two shard
     halves so the first half overlaps the second half's projection.
"""

import sys

if "/opt/trn_rl_repo" not in sys.path:
    sys.path.insert(0, "/opt/trn_rl_repo")

import numpy as np

B, S, D, R, N_IN, N_PROC, K = 4, 4096, 1024, 128, 8, 32, 4
N_CORES = 8
T_TOTAL = B * S
T = T_TOTAL // N_CORES   # tokens per core
G = T // 128             # token groups of 128 per core
KC = D // 128            # contraction chunks
HH_CHUNKS = [(0, 5), (5, 5), (10, 4), (14, 2)]  # (start, len) Householder chunks

_cache = {}
last_results = None


def _build_program():
    import concourse.bass as bass
    import concourse.mybir as mybir
    import concourse.tile as tile
    from concourse import bacc

    f32 = mybir.dt.float32
    f32r = mybir.dt.float32r
    f16 = mybir.dt.float16
    i32 = mybir.dt.int32
    mult = mybir.AluOpType.mult
    add = mybir.AluOpType.add
    sub = mybir.AluOpType.subtract
    X = mybir.AxisListType.X

    nc = bacc.Bacc(
        "TRN2",
        target_bir_lowering=False,
        debug=False,
        enable_asserts=False,
        num_devices=N_CORES,
    )

    x_d = nc.dram_tensor("x", [T, D], f32r, kind="ExternalInput").ap()
    iw_d = nc.dram_tensor("iw", [T, N_IN], f32, kind="ExternalInput").ap()
    idx_d = nc.dram_tensor("pidx", [T, K], i32, kind="ExternalInput").ap()
    wf_d = nc.dram_tensor("wflat", [D, R * N_IN], f16, kind="ExternalInput").ap()
    id_d = nc.dram_tensor("ident", [128, 128], f32r, kind="ExternalInput").ap()
    pp_d = nc.dram_tensor(
        "ppair", [N_PROC * N_PROC, 2 * R], f16, kind="ExternalInput"
    ).ap()
    out_d = nc.dram_tensor("out", [T, R], f32, kind="ExternalOutput").ap()

    x_re = x_d.rearrange("(g p) d -> p g d", p=128)       # [128, G, D]
    out_re = out_d.rearrange("(g p) r -> p g r", p=128)   # [128, G, R]

    with tile.TileContext(nc) as tc:
        with (
            tc.tile_pool(name="const", bufs=1) as cpool,
            tc.tile_pool(name="big", bufs=1) as bigpool,
            tc.tile_pool(name="xin", bufs=4) as xpool,
            tc.tile_pool(name="xt", bufs=4) as xtpool,
            tc.tile_pool(name="scl", bufs=4) as sclpool,
            tc.tile_pool(name="psxt", bufs=2, space="PSUM") as psA,
            tc.tile_pool(name="psproj", bufs=2, space="PSUM") as psB,
        ):
            # ---- constants / prefetches (small ones first; wf last so the
            # identity/indices don't queue behind its 2MB transfer) ----
            idm = cpool.tile([128, 128], f32r)
            nc.scalar.dma_start(idm[:], id_d[:])
            wf_sb = cpool.tile([128, KC, R * N_IN], f16)
            wf_re = wf_d.rearrange("(c p) m -> p c m", p=128)
            for c in range(KC):
                nc.scalar.dma_start(wf_sb[:, c:c + 1, :], wf_re[:, c:c + 1, :])
            iw_sb = cpool.tile([128, G, N_IN], f32)
            nc.scalar.dma_start(iw_sb[:], iw_d.rearrange("(g p) n -> p g n", p=128))
            idx_sb = cpool.tile([128, G, K], i32)
            nc.scalar.dma_start(idx_sb[:], idx_d.rearrange("(g p) k -> p g k", p=128))

            # pair indices j01 = 32*k0 + k1, j23 = 32*k2 + k3
            idx01 = cpool.tile([128, G], i32)
            idx23 = cpool.tile([128, G], i32)
            nc.vector.tensor_scalar(
                out=idx01[:], in0=idx_sb[:, :, 0], scalar1=N_PROC, scalar2=None,
                op0=mult,
            )
            nc.vector.tensor_tensor(
                out=idx01[:], in0=idx01[:], in1=idx_sb[:, :, 1], op=add
            )
            nc.vector.tensor_scalar(
                out=idx23[:], in0=idx_sb[:, :, 2], scalar1=N_PROC, scalar2=None,
                op0=mult,
            )
            nc.vector.tensor_tensor(
                out=idx23[:], in0=idx23[:], in1=idx_sb[:, :, 3], op=add
            )

            # gathered reflection vector pairs (already scaled by sqrt(2))
            v01 = bigpool.tile([128, G, 2 * R], f16)
            v23 = bigpool.tile([128, G, 2 * R], f16)
            for g in range(G):
                nc.gpsimd.indirect_dma_start(
                    out=v01[:, g, :],
                    out_offset=None,
                    in_=pp_d[:],
                    in_offset=bass.IndirectOffsetOnAxis(ap=idx01[:, g:g + 1], axis=0),
                )
                nc.gpsimd.indirect_dma_start(
                    out=v23[:, g, :],
                    out_offset=None,
                    in_=pp_d[:],
                    in_offset=bass.IndirectOffsetOnAxis(ap=idx23[:, g:g + 1], axis=0),
                )

            y_all = bigpool.tile([128, G, R], f16)
            hh_scr = bigpool.tile([128, G, R], f16)
            md = bigpool.tile([128, G], f16)
            mdb = bigpool.tile([128, 5, R], f16)
            v_views = [
                v01[:, :, 0:R],
                v01[:, :, R:2 * R],
                v23[:, :, 0:R],
                v23[:, :, R:2 * R],
            ]

            def hh_chunk(start, ln, tail=False):
                s = slice(start, start + ln)
                for k in range(K):
                    vk = v_views[k][:, s, :]
                    nc.vector.tensor_tensor(
                        out=hh_scr[:, s, :], in0=y_all[:, s, :], in1=vk, op=mult
                    )
                    with nc.allow_low_precision("fp16 Householder dots"):
                        nc.vector.tensor_reduce(
                            out=md[:, s], in_=hh_scr[:, s, :], axis=X, op=add
                        )
                    if tail:
                        # stay on DVE: cross-engine hops cost serial latency
                        nc.vector.tensor_tensor(
                            out=hh_scr[:, s, :],
                            in0=vk,
                            in1=md[:, s].unsqueeze(2).broadcast_to([128, ln, R]),
                            op=mult,
                        )
                    else:
                        # materialize the broadcast on ScalarE so the DVE
                        # multiply keeps its unit-stride 2x mode
                        nc.scalar.copy(
                            mdb[:, 0:ln, :],
                            md[:, s].unsqueeze(2).broadcast_to([128, ln, R]),
                        )
                        nc.vector.tensor_tensor(
                            out=hh_scr[:, s, :], in0=vk, in1=mdb[:, 0:ln, :], op=mult
                        )
                    nc.vector.tensor_tensor(
                        out=y_all[:, s, :],
                        in0=y_all[:, s, :],
                        in1=hh_scr[:, s, :],
                        op=sub,
                    )
                y32 = sclpool.tile([128, ln, R], f32, tag="y32")
                nc.vector.tensor_copy(y32[:], y_all[:, s, :])
                nc.sync.dma_start(out_re[:, s, :], y32[:])

            # ---- projection + weighted sum ----
            for g in range(G):
                x_g = xpool.tile([128, D], f32r, tag="x")
                nc.sync.dma_start(x_g[:], x_re[:, g, :])

                ps_xt = psA.tile([128, KC, 128], f32r, tag="psxt")
                for c in range(KC):
                    nc.tensor.transpose(
                        ps_xt[:, c, :], x_g[:, c * 128:(c + 1) * 128], idm[:]
                    )
                xt_g = xtpool.tile([128, KC, 128], f16, tag="xt")
                nc.scalar.copy(xt_g[:], ps_xt[:])

                ps_proj = psB.tile([128, R, N_IN], f32, tag="psproj")
                for c in range(KC):
                    for h in range(2):
                        nc.tensor.matmul(
                            ps_proj[:, 64 * h:64 * h + 64, :],
                            lhsT=xt_g[:, c, :],
                            rhs=wf_sb[:, c, h * 512:(h + 1) * 512],
                            start=(c == 0),
                            stop=(c == KC - 1),
                        )

                scaled = sclpool.tile([128, R, N_IN], f32, tag="scl")
                nc.vector.tensor_tensor(
                    out=scaled[:],
                    in0=ps_proj[:],
                    in1=iw_sb[:, g].unsqueeze(1).broadcast_to([128, R, N_IN]),
                    op=mult,
                )
                with nc.allow_low_precision("fp16 y for Householder"):
                    nc.vector.tensor_reduce(
                        out=y_all[:, g, :], in_=scaled[:], axis=X, op=add
                    )

                for cs, cl in HH_CHUNKS[:-1]:
                    if g == cs + cl - 1:
                        hh_chunk(cs, cl)

            hh_chunk(*HH_CHUNKS[-1], tail=True)

    nc.compile()
    return nc


def _get_program():
    if "nc" not in _cache:
        _cache["nc"] = _build_program()
    return _cache["nc"]


def _host_prep(x, input_weights, process_indices, input_neurons, process_neurons):
    xf = np.ascontiguousarray(np.asarray(x, dtype=np.float32)).reshape(T_TOTAL, D)
    iwf = np.ascontiguousarray(np.asarray(input_weights, dtype=np.float32)).reshape(
        T_TOTAL, N_IN
    )
    idxf = np.ascontiguousarray(np.asarray(process_indices, dtype=np.int32)).reshape(
        T_TOTAL, K
    )
    # W layout: wflat[d, r*8+n] = input_neurons[n, d, r]  (r-major, n innermost)
    wflat = np.ascontiguousarray(
        np.transpose(np.asarray(input_neurons, dtype=np.float32), (1, 2, 0)).reshape(
            D, R * N_IN
        )
    ).astype(np.float16)
    ident = np.eye(128, dtype=np.float32)
    # pair table: row j0*32+j1 = sqrt(2) * [P[j0] | P[j1]]
    # (sqrt(2) scaling turns y - (v*sqrt2)((v*sqrt2).y) into y - 2 v (v.y))
    pn = np.asarray(process_neurons, dtype=np.float32) * np.float32(np.sqrt(2.0))
    ppair = np.concatenate(
        [
            np.repeat(pn, N_PROC, axis=0),
            np.tile(pn, (N_PROC, 1)),
        ],
        axis=1,
    ).astype(np.float16)
    in_maps = []
    for c in range(N_CORES):
        sl = slice(c * T, (c + 1) * T)
        in_maps.append(
            {
                "x": xf[sl],
                "iw": iwf[sl],
                "pidx": idxf[sl],
                "wflat": wflat,
                "ident": ident,
                "ppair": ppair,
            }
        )
    return in_maps


def kernel(x, input_weights, process_indices, input_neurons, process_neurons):
    global last_results
    from concourse.bass_utils import run_bass_kernel_spmd

    nc = _get_program()
    in_maps = _host_prep(
        x, input_weights, process_indices, input_neurons, process_neurons
    )
    res = run_bass_kernel_spmd(nc, in_maps, core_ids=list(range(N_CORES)))
    last_results = res
    out = np.concatenate([res.results[c]["out"] for c in range(N_CORES)], axis=0)
    return out.reshape(B, S, R)


# revision 31
# speedup vs baseline: 1.1456x; 1.0174x over previous
"""Trainium2 Bass kernel for nn_NeuronCircuitDown (moe_routing).

Math (per token t):
  y[t, :]  = sum_n iw[t, n] * (x[t, :] @ Wn[n])          # projection, Wn: [D, R]
  then K=4 sequential Householder reflections with vectors gathered from a
  32-row table by process_indices:
  y <- y - 2 * v * (v . y)   (table rows are unit-norm up to 1e-7, so the
                              reference's re-normalization is a no-op at
                              fp32 precision and is skipped)

Distribution: data-parallel over the 16384 tokens, 2048 tokens per core on 8
cores; weights/table replicated (weights pre-cast to fp16 / laid out on the
host — parameter prep only; all per-token compute happens on device).

Per-core device pipeline (tokens on partitions, 16 groups of 128):
  1. x loaded fp32 (Sync HWDGE), transposed per 128x128 block on the PE
     (fp32r transpose mode), evacuated PSUM->SBUF with an fp16 cast on
     ScalarE.  PE alternates transposes and matmuls so it never idles
     (keeps the HAM clock-gate warm); no DMA-xbar transposes (each one
     acts as a global DMA barrier on this hardware).
  2. fp16 matmuls: proj[t, r*8+n] accumulated over 8 K-chunks into PSUM
     (r-major layout so the n-reduction is contiguous)
  3. weighted sum on DVE: broadcast multiply (stride-0 AP) + contiguous
     reduction over n -> y fp32
  4. Householder: vectors pre-gathered via indirect DMA from a host-built
     pair table (rows = sqrt(2)*[P[j0] | P[j1]], indexed by j0*32+j1);
     batched multiply/reduce/multiply/subtract on DVE, run in two shard
     halves so the first half overlaps the second half's projection.
"""

import sys

if "/opt/trn_rl_repo" not in sys.path:
    sys.path.insert(0, "/opt/trn_rl_repo")

import numpy as np

B, S, D, R, N_IN, N_PROC, K = 4, 4096, 1024, 128, 8, 32, 4
N_CORES = 8
T_TOTAL = B * S
T = T_TOTAL // N_CORES   # tokens per core
G = T // 128             # token groups of 128 per core
KC = D // 128            # contraction chunks
HH_CHUNKS = [(0, 5), (5, 5), (10, 4), (14, 2)]  # (start, len) Householder chunks

_cache = {}
last_results = None


def _build_program():
    import concourse.bass as bass
    import concourse.mybir as mybir
    import concourse.tile as tile
    from concourse import bacc

    f32 = mybir.dt.float32
    f32r = mybir.dt.float32r
    f16 = mybir.dt.float16
    i32 = mybir.dt.int32
    mult = mybir.AluOpType.mult
    add = mybir.AluOpType.add
    sub = mybir.AluOpType.subtract
    X = mybir.AxisListType.X

    nc = bacc.Bacc(
        "TRN2",
        target_bir_lowering=False,
        debug=False,
        enable_asserts=False,
        num_devices=N_CORES,
    )

    x_d = nc.dram_tensor("x", [T, D], f32r, kind="ExternalInput").ap()
    iw_d = nc.dram_tensor("iw", [T, N_IN], f32, kind="ExternalInput").ap()
    idx_d = nc.dram_tensor("pidx", [T, K], i32, kind="ExternalInput").ap()
    wf_d = nc.dram_tensor("wflat", [D, R * N_IN], f16, kind="ExternalInput").ap()
    id_d = nc.dram_tensor("ident", [128, 128], f32r, kind="ExternalInput").ap()
    pp_d = nc.dram_tensor(
        "ppair", [N_PROC * N_PROC, 2 * R], f16, kind="ExternalInput"
    ).ap()
    out_d = nc.dram_tensor("out", [T, R], f32, kind="ExternalOutput").ap()

    x_re = x_d.rearrange("(g p) d -> p g d", p=128)       # [128, G, D]
    out_re = out_d.rearrange("(g p) r -> p g r", p=128)   # [128, G, R]

    with tile.TileContext(nc) as tc:
        with (
            tc.tile_pool(name="const", bufs=1) as cpool,
            tc.tile_pool(name="big", bufs=1) as bigpool,
            tc.tile_pool(name="xin", bufs=4) as xpool,
            tc.tile_pool(name="xt", bufs=4) as xtpool,
            tc.tile_pool(name="scl", bufs=4) as sclpool,
            tc.tile_pool(name="psxt", bufs=2, space="PSUM") as psA,
            tc.tile_pool(name="psproj", bufs=2, space="PSUM") as psB,
        ):
            # ---- constants / prefetches (small ones first; wf last so the
            # identity/indices don't queue behind its 2MB transfer) ----
            idm = cpool.tile([128, 128], f32r)
            nc.scalar.dma_start(idm[:], id_d[:])
            wf_sb = cpool.tile([128, KC, R * N_IN], f16)
            wf_re = wf_d.rearrange("(c p) m -> p c m", p=128)
            for c in range(KC):
                nc.scalar.dma_start(wf_sb[:, c:c + 1, :], wf_re[:, c:c + 1, :])
            iw_sb = cpool.tile([128, G, N_IN], f32)
            nc.scalar.dma_start(iw_sb[:], iw_d.rearrange("(g p) n -> p g n", p=128))
            idx_sb = cpool.tile([128, G, K], i32)
            nc.scalar.dma_start(idx_sb[:], idx_d.rearrange("(g p) k -> p g k", p=128))

            # pair indices j01 = 32*k0 + k1, j23 = 32*k2 + k3
            idx01 = cpool.tile([128, G], i32)
            idx23 = cpool.tile([128, G], i32)
            nc.vector.tensor_scalar(
                out=idx01[:], in0=idx_sb[:, :, 0], scalar1=N_PROC, scalar2=None,
                op0=mult,
            )
            nc.vector.tensor_tensor(
                out=idx01[:], in0=idx01[:], in1=idx_sb[:, :, 1], op=add
            )
            nc.vector.tensor_scalar(
                out=idx23[:], in0=idx_sb[:, :, 2], scalar1=N_PROC, scalar2=None,
                op0=mult,
            )
            nc.vector.tensor_tensor(
                out=idx23[:], in0=idx23[:], in1=idx_sb[:, :, 3], op=add
            )

            # gathered reflection vector pairs (already scaled by sqrt(2))
            v01 = bigpool.tile([128, G, 2 * R], f16)
            v23 = bigpool.tile([128, G, 2 * R], f16)
            for g in range(G):
                nc.gpsimd.indirect_dma_start(
                    out=v01[:, g, :],
                    out_offset=None,
                    in_=pp_d[:],
                    in_offset=bass.IndirectOffsetOnAxis(ap=idx01[:, g:g + 1], axis=0),
                )
                nc.gpsimd.indirect_dma_start(
                    out=v23[:, g, :],
                    out_offset=None,
                    in_=pp_d[:],
                    in_offset=bass.IndirectOffsetOnAxis(ap=idx23[:, g:g + 1], axis=0),
                )

            y_all = bigpool.tile([128, G, R], f16)
            hh_scr = bigpool.tile([128, G, R], f16)
            md = bigpool.tile([128, G], f16)
            mdb = bigpool.tile([128, 5, R], f16)
            v_views = [
                v01[:, :, 0:R],
                v01[:, :, R:2 * R],
                v23[:, :, 0:R],
                v23[:, :, R:2 * R],
            ]

            def hh_chunk(start, ln, tail=False):
                s = slice(start, start + ln)
                for k in range(K):
                    vk = v_views[k][:, s, :]
                    nc.vector.tensor_tensor(
                        out=hh_scr[:, s, :], in0=y_all[:, s, :], in1=vk, op=mult
                    )
                    with nc.allow_low_precision("fp16 Householder dots"):
                        nc.vector.tensor_reduce(
                            out=md[:, s], in_=hh_scr[:, s, :], axis=X, op=add
                        )
                    if tail:
                        # stay on DVE: cross-engine hops cost serial latency
                        nc.vector.tensor_tensor(
                            out=hh_scr[:, s, :],
                            in0=vk,
                            in1=md[:, s].unsqueeze(2).broadcast_to([128, ln, R]),
                            op=mult,
                        )
                    else:
                        # materialize the broadcast on ScalarE so the DVE
                        # multiply keeps its unit-stride 2x mode
                        nc.scalar.copy(
                            mdb[:, 0:ln, :],
                            md[:, s].unsqueeze(2).broadcast_to([128, ln, R]),
                        )
                        nc.vector.tensor_tensor(
                            out=hh_scr[:, s, :], in0=vk, in1=mdb[:, 0:ln, :], op=mult
                        )
                    nc.vector.tensor_tensor(
                        out=y_all[:, s, :],
                        in0=y_all[:, s, :],
                        in1=hh_scr[:, s, :],
                        op=sub,
                    )
                y32 = sclpool.tile([128, ln, R], f32, tag="y32")
                nc.vector.tensor_copy(y32[:], y_all[:, s, :])
                nc.sync.dma_start(out_re[:, s, :], y32[:])

            # ---- projection + weighted sum ----
            for g in range(G):
                x_g = xpool.tile([128, D], f32r, tag="x")
                nc.sync.dma_start(x_g[:], x_re[:, g, :])

                ps_xt = psA.tile([128, KC, 128], f32r, tag="psxt")
                for c in range(KC):
                    nc.tensor.transpose(
                        ps_xt[:, c, :], x_g[:, c * 128:(c + 1) * 128], idm[:]
                    )
                xt_g = xtpool.tile([128, KC, 128], f16, tag="xt")
                nc.scalar.copy(xt_g[:], ps_xt[:])

                ps_proj = psB.tile([128, R, N_IN], f32, tag="psproj")
                for c in range(KC):
                    for h in range(2):
                        nc.tensor.matmul(
                            ps_proj[:, 64 * h:64 * h + 64, :],
                            lhsT=xt_g[:, c, :],
                            rhs=wf_sb[:, c, h * 512:(h + 1) * 512],
                            start=(c == 0),
                            stop=(c == KC - 1),
                        )

                scaled = sclpool.tile([128, R, N_IN], f16, tag="scl")
                with nc.allow_low_precision("fp16 weighted-sum tree"):
                    nc.vector.tensor_tensor(
                        out=scaled[:],
                        in0=ps_proj[:],
                        in1=iw_sb[:, g].unsqueeze(1).broadcast_to([128, R, N_IN]),
                        op=mult,
                    )
                    t1 = sclpool.tile([128, R, 4], f16, tag="t1")
                    nc.vector.tensor_tensor(
                        out=t1[:], in0=scaled[:, :, 0:4], in1=scaled[:, :, 4:8],
                        op=add,
                    )
                    t2 = sclpool.tile([128, R, 2], f16, tag="t2")
                    nc.vector.tensor_tensor(
                        out=t2[:], in0=t1[:, :, 0:2], in1=t1[:, :, 2:4], op=add
                    )
                    nc.vector.tensor_tensor(
                        out=y_all[:, g, :], in0=t2[:, :, 0], in1=t2[:, :, 1], op=add
                    )

                for cs, cl in HH_CHUNKS[:-1]:
                    if g == cs + cl - 1:
                        hh_chunk(cs, cl)

            hh_chunk(*HH_CHUNKS[-1], tail=True)

    nc.compile()
    return nc


def _get_program():
    if "nc" not in _cache:
        _cache["nc"] = _build_program()
    return _cache["nc"]


def _host_prep(x, input_weights, process_indices, input_neurons, process_neurons):
    xf = np.ascontiguousarray(np.asarray(x, dtype=np.float32)).reshape(T_TOTAL, D)
    iwf = np.ascontiguousarray(np.asarray(input_weights, dtype=np.float32)).reshape(
        T_TOTAL, N_IN
    )
    idxf = np.ascontiguousarray(np.asarray(process_indices, dtype=np.int32)).reshape(
        T_TOTAL, K
    )
    # W layout: wflat[d, r*8+n] = input_neurons[n, d, r]  (r-major, n innermost)
    wflat = np.ascontiguousarray(
        np.transpose(np.asarray(input_neurons, dtype=np.float32), (1, 2, 0)).reshape(
            D, R * N_IN
        )
    ).astype(np.float16)
    ident = np.eye(128, dtype=np.float32)
    # pair table: row j0*32+j1 = sqrt(2) * [P[j0] | P[j1]]
    # (sqrt(2) scaling turns y - (v*sqrt2)((v*sqrt2).y) into y - 2 v (v.y))
    pn = np.asarray(process_neurons, dtype=np.float32) * np.float32(np.sqrt(2.0))
    ppair = np.concatenate(
        [
            np.repeat(pn, N_PROC, axis=0),
            np.tile(pn, (N_PROC, 1)),
        ],
        axis=1,
    ).astype(np.float16)
    in_maps = []
    for c in range(N_CORES):
        sl = slice(c * T, (c + 1) * T)
        in_maps.append(
            {
                "x": xf[sl],
                "iw": iwf[sl],
                "pidx": idxf[sl],
                "wflat": wflat,
                "ident": ident,
                "ppair": ppair,
            }
        )
    return in_maps


def kernel(x, input_weights, process_indices, input_neurons, process_neurons):
    global last_results
    from concourse.bass_utils import run_bass_kernel_spmd

    nc = _get_program()
    in_maps = _host_prep(
        x, input_weights, process_indices, input_neurons, process_neurons
    )
    res = run_bass_kernel_spmd(nc, in_maps, core_ids=list(range(N_CORES)))
    last_results = res
    out = np.concatenate([res.results[c]["out"] for c in range(N_CORES)], axis=0)
    return out.reshape(B, S, R)


# revision 32
# speedup vs baseline: 1.1911x; 1.0397x over previous
"""Trainium2 Bass kernel for nn_NeuronCircuitDown (moe_routing).

Math (per token t):
  y[t, :]  = sum_n iw[t, n] * (x[t, :] @ Wn[n])          # projection, Wn: [D, R]
  then K=4 sequential Householder reflections with vectors gathered from a
  32-row table by process_indices:
  y <- y - 2 * v * (v . y)   (table rows are unit-norm up to 1e-7, so the
                              reference's re-normalization is a no-op at
                              fp32 precision and is skipped)

Distribution: data-parallel over the 16384 tokens, 2048 tokens per core on 8
cores; weights/table replicated (weights pre-cast to fp16 / laid out on the
host — parameter prep only; all per-token compute happens on device).

Per-core device pipeline (tokens on partitions, 16 groups of 128):
  1. x loaded fp32 (Sync HWDGE), transposed per 128x128 block on the PE
     (fp32r transpose mode), evacuated PSUM->SBUF with an fp16 cast on
     ScalarE.  PE alternates transposes and matmuls so it never idles
     (keeps the HAM clock-gate warm); no DMA-xbar transposes (each one
     acts as a global DMA barrier on this hardware).
  2. fp16 matmuls: proj[t, r*8+n] accumulated over 8 K-chunks into PSUM
     (r-major layout so the n-reduction is contiguous)
  3. weighted sum on DVE: broadcast multiply (stride-0 AP) + contiguous
     reduction over n -> y fp32
  4. Householder: vectors pre-gathered via indirect DMA from a host-built
     pair table (rows = sqrt(2)*[P[j0] | P[j1]], indexed by j0*32+j1);
     batched multiply/reduce/multiply/subtract on DVE, run in two shard
     halves so the first half overlaps the second half's projection.
"""

import sys

if "/opt/trn_rl_repo" not in sys.path:
    sys.path.insert(0, "/opt/trn_rl_repo")

import numpy as np

B, S, D, R, N_IN, N_PROC, K = 4, 4096, 1024, 128, 8, 32, 4
N_CORES = 8
T_TOTAL = B * S
T = T_TOTAL // N_CORES   # tokens per core
G = T // 128             # token groups of 128 per core
KC = D // 128            # contraction chunks
HH_CHUNKS = [(0, 5), (5, 5), (10, 4), (14, 2)]  # (start, len) Householder chunks

_cache = {}
last_results = None


def _build_program():
    import concourse.bass as bass
    import concourse.mybir as mybir
    import concourse.tile as tile
    from concourse import bacc

    f32 = mybir.dt.float32
    f32r = mybir.dt.float32r
    f16 = mybir.dt.float16
    i32 = mybir.dt.int32
    mult = mybir.AluOpType.mult
    add = mybir.AluOpType.add
    sub = mybir.AluOpType.subtract
    X = mybir.AxisListType.X

    nc = bacc.Bacc(
        "TRN2",
        target_bir_lowering=False,
        debug=False,
        enable_asserts=False,
        num_devices=N_CORES,
    )

    x_d = nc.dram_tensor("x", [T, D], f32r, kind="ExternalInput").ap()
    iw_d = nc.dram_tensor("iw", [T, N_IN], f32, kind="ExternalInput").ap()
    idx_d = nc.dram_tensor("pidx", [T, K], i32, kind="ExternalInput").ap()
    wf_d = nc.dram_tensor("wflat", [D, R * N_IN], f16, kind="ExternalInput").ap()
    id_d = nc.dram_tensor("ident", [128, 128], f32r, kind="ExternalInput").ap()
    pp_d = nc.dram_tensor(
        "ppair", [N_PROC * N_PROC, 2 * R], f16, kind="ExternalInput"
    ).ap()
    out_d = nc.dram_tensor("out", [T, R], f32, kind="ExternalOutput").ap()

    x_re = x_d.rearrange("(g p) d -> p g d", p=128)       # [128, G, D]
    out_re = out_d.rearrange("(g p) r -> p g r", p=128)   # [128, G, R]

    with tile.TileContext(nc) as tc:
        with (
            tc.tile_pool(name="const", bufs=1) as cpool,
            tc.tile_pool(name="big", bufs=1) as bigpool,
            tc.tile_pool(name="xin", bufs=6) as xpool,
            tc.tile_pool(name="xt", bufs=4) as xtpool,
            tc.tile_pool(name="scl", bufs=4) as sclpool,
            tc.tile_pool(name="psxt", bufs=4, space="PSUM") as psA,
            tc.tile_pool(name="psproj", bufs=2, space="PSUM") as psB,
        ):
            # ---- constants / prefetches (small ones first; wf last so the
            # identity/indices don't queue behind its 2MB transfer) ----
            idm = cpool.tile([128, 128], f32r)
            nc.scalar.dma_start(idm[:], id_d[:])
            wf_sb = cpool.tile([128, KC, R * N_IN], f16)
            wf_re = wf_d.rearrange("(c p) m -> p c m", p=128)
            for c in range(KC):
                nc.scalar.dma_start(wf_sb[:, c:c + 1, :], wf_re[:, c:c + 1, :])
            iw_sb = cpool.tile([128, G, N_IN], f32)
            nc.scalar.dma_start(iw_sb[:], iw_d.rearrange("(g p) n -> p g n", p=128))
            idx_sb = cpool.tile([128, G, K], i32)
            nc.scalar.dma_start(idx_sb[:], idx_d.rearrange("(g p) k -> p g k", p=128))

            # pair indices j01 = 32*k0 + k1, j23 = 32*k2 + k3
            idx01 = cpool.tile([128, G], i32)
            idx23 = cpool.tile([128, G], i32)
            nc.vector.tensor_scalar(
                out=idx01[:], in0=idx_sb[:, :, 0], scalar1=N_PROC, scalar2=None,
                op0=mult,
            )
            nc.vector.tensor_tensor(
                out=idx01[:], in0=idx01[:], in1=idx_sb[:, :, 1], op=add
            )
            nc.vector.tensor_scalar(
                out=idx23[:], in0=idx_sb[:, :, 2], scalar1=N_PROC, scalar2=None,
                op0=mult,
            )
            nc.vector.tensor_tensor(
                out=idx23[:], in0=idx23[:], in1=idx_sb[:, :, 3], op=add
            )

            # gathered reflection vector pairs (already scaled by sqrt(2))
            v01 = bigpool.tile([128, G, 2 * R], f16)
            v23 = bigpool.tile([128, G, 2 * R], f16)
            for g in range(G):
                nc.gpsimd.indirect_dma_start(
                    out=v01[:, g, :],
                    out_offset=None,
                    in_=pp_d[:],
                    in_offset=bass.IndirectOffsetOnAxis(ap=idx01[:, g:g + 1], axis=0),
                )
                nc.gpsimd.indirect_dma_start(
                    out=v23[:, g, :],
                    out_offset=None,
                    in_=pp_d[:],
                    in_offset=bass.IndirectOffsetOnAxis(ap=idx23[:, g:g + 1], axis=0),
                )

            y_all = bigpool.tile([128, G, R], f16)
            hh_scr = bigpool.tile([128, G, R], f16)
            md = bigpool.tile([128, G], f16)
            mdb = bigpool.tile([128, 5, R], f16)
            v_views = [
                v01[:, :, 0:R],
                v01[:, :, R:2 * R],
                v23[:, :, 0:R],
                v23[:, :, R:2 * R],
            ]

            def hh_chunk(start, ln, tail=False):
                s = slice(start, start + ln)
                for k in range(K):
                    vk = v_views[k][:, s, :]
                    nc.vector.tensor_tensor(
                        out=hh_scr[:, s, :], in0=y_all[:, s, :], in1=vk, op=mult
                    )
                    with nc.allow_low_precision("fp16 Householder dots"):
                        nc.vector.tensor_reduce(
                            out=md[:, s], in_=hh_scr[:, s, :], axis=X, op=add
                        )
                    if tail:
                        # stay on DVE: cross-engine hops cost serial latency
                        nc.vector.tensor_tensor(
                            out=hh_scr[:, s, :],
                            in0=vk,
                            in1=md[:, s].unsqueeze(2).broadcast_to([128, ln, R]),
                            op=mult,
                        )
                    else:
                        # materialize the broadcast on ScalarE so the DVE
                        # multiply keeps its unit-stride 2x mode
                        nc.scalar.copy(
                            mdb[:, 0:ln, :],
                            md[:, s].unsqueeze(2).broadcast_to([128, ln, R]),
                        )
                        nc.vector.tensor_tensor(
                            out=hh_scr[:, s, :], in0=vk, in1=mdb[:, 0:ln, :], op=mult
                        )
                    nc.vector.tensor_tensor(
                        out=y_all[:, s, :],
                        in0=y_all[:, s, :],
                        in1=hh_scr[:, s, :],
                        op=sub,
                    )
                y32 = sclpool.tile([128, ln, R], f32, tag="y32")
                nc.vector.tensor_copy(y32[:], y_all[:, s, :])
                nc.sync.dma_start(out_re[:, s, :], y32[:])

            # ---- projection + weighted sum ----
            for g in range(G):
                x_g = xpool.tile([128, D], f32r, tag="x")
                nc.sync.dma_start(x_g[:], x_re[:, g, :])

                xt_g = xtpool.tile([128, KC, 128], f16, tag="xt")
                for half in range(2):
                    ps_xt = psA.tile([128, KC // 2, 128], f32r, tag="psxt")
                    for cc in range(KC // 2):
                        c = half * (KC // 2) + cc
                        nc.tensor.transpose(
                            ps_xt[:, cc, :], x_g[:, c * 128:(c + 1) * 128], idm[:]
                        )
                    nc.scalar.copy(
                        xt_g[:, half * (KC // 2):(half + 1) * (KC // 2), :],
                        ps_xt[:],
                    )

                ps_proj = psB.tile([128, R, N_IN], f32, tag="psproj")
                for c in range(KC):
                    for h in range(2):
                        nc.tensor.matmul(
                            ps_proj[:, 64 * h:64 * h + 64, :],
                            lhsT=xt_g[:, c, :],
                            rhs=wf_sb[:, c, h * 512:(h + 1) * 512],
                            start=(c == 0),
                            stop=(c == KC - 1),
                        )

                scaled = sclpool.tile([128, R, N_IN], f16, tag="scl")
                with nc.allow_low_precision("fp16 weighted-sum tree"):
                    nc.vector.tensor_tensor(
                        out=scaled[:],
                        in0=ps_proj[:],
                        in1=iw_sb[:, g].unsqueeze(1).broadcast_to([128, R, N_IN]),
                        op=mult,
                    )
                    t1 = sclpool.tile([128, R, 4], f16, tag="t1")
                    nc.vector.tensor_tensor(
                        out=t1[:], in0=scaled[:, :, 0:4], in1=scaled[:, :, 4:8],
                        op=add,
                    )
                    t2 = sclpool.tile([128, R, 2], f16, tag="t2")
                    nc.vector.tensor_tensor(
                        out=t2[:], in0=t1[:, :, 0:2], in1=t1[:, :, 2:4], op=add
                    )
                    nc.vector.tensor_tensor(
                        out=y_all[:, g, :], in0=t2[:, :, 0], in1=t2[:, :, 1], op=add
                    )

                for cs, cl in HH_CHUNKS[:-1]:
                    if g == cs + cl - 1:
                        hh_chunk(cs, cl)

            hh_chunk(*HH_CHUNKS[-1], tail=True)

    nc.compile()
    return nc


def _get_program():
    if "nc" not in _cache:
        _cache["nc"] = _build_program()
    return _cache["nc"]


def _host_prep(x, input_weights, process_indices, input_neurons, process_neurons):
    xf = np.ascontiguousarray(np.asarray(x, dtype=np.float32)).reshape(T_TOTAL, D)
    iwf = np.ascontiguousarray(np.asarray(input_weights, dtype=np.float32)).reshape(
        T_TOTAL, N_IN
    )
    idxf = np.ascontiguousarray(np.asarray(process_indices, dtype=np.int32)).reshape(
        T_TOTAL, K
    )
    # W layout: wflat[d, r*8+n] = input_neurons[n, d, r]  (r-major, n innermost)
    wflat = np.ascontiguousarray(
        np.transpose(np.asarray(input_neurons, dtype=np.float32), (1, 2, 0)).reshape(
            D, R * N_IN
        )
    ).astype(np.float16)
    ident = np.eye(128, dtype=np.float32)
    # pair table: row j0*32+j1 = sqrt(2) * [P[j0] | P[j1]]
    # (sqrt(2) scaling turns y - (v*sqrt2)((v*sqrt2).y) into y - 2 v (v.y))
    pn = np.asarray(process_neurons, dtype=np.float32) * np.float32(np.sqrt(2.0))
    ppair = np.concatenate(
        [
            np.repeat(pn, N_PROC, axis=0),
            np.tile(pn, (N_PROC, 1)),
        ],
        axis=1,
    ).astype(np.float16)
    in_maps = []
    for c in range(N_CORES):
        sl = slice(c * T, (c + 1) * T)
        in_maps.append(
            {
                "x": xf[sl],
                "iw": iwf[sl],
                "pidx": idxf[sl],
                "wflat": wflat,
                "ident": ident,
                "ppair": ppair,
            }
        )
    return in_maps


def kernel(x, input_weights, process_indices, input_neurons, process_neurons):
    global last_results
    from concourse.bass_utils import run_bass_kernel_spmd

    nc = _get_program()
    in_maps = _host_prep(
        x, input_weights, process_indices, input_neurons, process_neurons
    )
    res = run_bass_kernel_spmd(nc, in_maps, core_ids=list(range(N_CORES)))
    last_results = res
    out = np.concatenate([res.results[c]["out"] for c in range(N_CORES)], axis=0)
    return out.reshape(B, S, R)


# revision 33
# speedup vs baseline: 1.2163x; 1.0212x over previous
"""Trainium2 Bass kernel for nn_NeuronCircuitDown (moe_routing).

Math (per token t):
  y[t, :]  = sum_n iw[t, n] * (x[t, :] @ Wn[n])          # projection, Wn: [D, R]
  then K=4 sequential Householder reflections with vectors gathered from a
  32-row table by process_indices:
  y <- y - 2 * v * (v . y)   (table rows are unit-norm up to 1e-7, so the
                              reference's re-normalization is a no-op at
                              fp32 precision and is skipped)

Distribution: data-parallel over the 16384 tokens, 2048 tokens per core on 8
cores; weights/table replicated (weights pre-cast to fp16 / laid out on the
host — parameter prep only; all per-token compute happens on device).

Per-core device pipeline (tokens on partitions, 16 groups of 128):
  1. x loaded fp32 (Sync HWDGE), transposed per 128x128 block on the PE
     (fp32r transpose mode), evacuated PSUM->SBUF with an fp16 cast on
     ScalarE.  PE alternates transposes and matmuls so it never idles
     (keeps the HAM clock-gate warm); no DMA-xbar transposes (each one
     acts as a global DMA barrier on this hardware).
  2. fp16 matmuls: proj[t, r*8+n] accumulated over 8 K-chunks into PSUM
     (r-major layout so the n-reduction is contiguous)
  3. weighted sum on DVE: broadcast multiply (stride-0 AP) + contiguous
     reduction over n -> y fp32
  4. Householder: vectors pre-gathered via indirect DMA from a host-built
     pair table (rows = sqrt(2)*[P[j0] | P[j1]], indexed by j0*32+j1);
     batched multiply/reduce/multiply/subtract on DVE, run in two shard
     halves so the first half overlaps the second half's projection.
"""

import sys

if "/opt/trn_rl_repo" not in sys.path:
    sys.path.insert(0, "/opt/trn_rl_repo")

import numpy as np

B, S, D, R, N_IN, N_PROC, K = 4, 4096, 1024, 128, 8, 32, 4
N_CORES = 8
T_TOTAL = B * S
T = T_TOTAL // N_CORES   # tokens per core
G = T // 128             # token groups of 128 per core
KC = D // 128            # contraction chunks
HH_CHUNKS = [(0, 5), (5, 5), (10, 4), (14, 2)]  # (start, len) Householder chunks

_cache = {}
last_results = None


def _build_program():
    import concourse.bass as bass
    import concourse.mybir as mybir
    import concourse.tile as tile
    from concourse import bacc

    f32 = mybir.dt.float32
    f32r = mybir.dt.float32r
    f16 = mybir.dt.float16
    i32 = mybir.dt.int32
    mult = mybir.AluOpType.mult
    add = mybir.AluOpType.add
    sub = mybir.AluOpType.subtract
    X = mybir.AxisListType.X

    nc = bacc.Bacc(
        "TRN2",
        target_bir_lowering=False,
        debug=False,
        enable_asserts=False,
        num_devices=N_CORES,
    )

    x_d = nc.dram_tensor("x", [T, D], f32r, kind="ExternalInput").ap()
    iw_d = nc.dram_tensor("iw", [T, N_IN], f32, kind="ExternalInput").ap()
    idx_d = nc.dram_tensor("pidx", [T, K], i32, kind="ExternalInput").ap()
    wf_d = nc.dram_tensor("wflat", [D, R * N_IN], f16, kind="ExternalInput").ap()
    id_d = nc.dram_tensor("ident", [128, 128], f32r, kind="ExternalInput").ap()
    pp_d = nc.dram_tensor(
        "ppair", [N_PROC * N_PROC, 2 * R], f16, kind="ExternalInput"
    ).ap()
    out_d = nc.dram_tensor("out", [T, R], f32, kind="ExternalOutput").ap()

    x_re = x_d.rearrange("(g p) d -> p g d", p=128)       # [128, G, D]
    out_re = out_d.rearrange("(g p) r -> p g r", p=128)   # [128, G, R]

    with tile.TileContext(nc) as tc:
        with (
            tc.tile_pool(name="const", bufs=1) as cpool,
            tc.tile_pool(name="big", bufs=1) as bigpool,
            tc.tile_pool(name="xin", bufs=6) as xpool,
            tc.tile_pool(name="xt", bufs=4) as xtpool,
            tc.tile_pool(name="scl", bufs=4) as sclpool,
            tc.tile_pool(name="psxt", bufs=4, space="PSUM") as psA,
            tc.tile_pool(name="psproj", bufs=2, space="PSUM") as psB,
        ):
            # ---- constants / prefetches (small ones first; wf last so the
            # identity/indices don't queue behind its 2MB transfer) ----
            idm = cpool.tile([128, 128], f32r)
            nc.scalar.dma_start(idm[:], id_d[:])
            wf_sb = cpool.tile([128, KC, R * N_IN], f16)
            wf_re = wf_d.rearrange("(c p) m -> p c m", p=128)
            for c in range(KC):
                nc.scalar.dma_start(wf_sb[:, c:c + 1, :], wf_re[:, c:c + 1, :])
            iw_sb = cpool.tile([128, G, N_IN], f32)
            nc.scalar.dma_start(iw_sb[:], iw_d.rearrange("(g p) n -> p g n", p=128))
            idx_sb = cpool.tile([128, G, K], i32)
            nc.scalar.dma_start(idx_sb[:], idx_d.rearrange("(g p) k -> p g k", p=128))

            # pair indices j01 = 32*k0 + k1, j23 = 32*k2 + k3
            idx01 = cpool.tile([128, G], i32)
            idx23 = cpool.tile([128, G], i32)
            nc.vector.tensor_scalar(
                out=idx01[:], in0=idx_sb[:, :, 0], scalar1=N_PROC, scalar2=None,
                op0=mult,
            )
            nc.vector.tensor_tensor(
                out=idx01[:], in0=idx01[:], in1=idx_sb[:, :, 1], op=add
            )
            nc.vector.tensor_scalar(
                out=idx23[:], in0=idx_sb[:, :, 2], scalar1=N_PROC, scalar2=None,
                op0=mult,
            )
            nc.vector.tensor_tensor(
                out=idx23[:], in0=idx23[:], in1=idx_sb[:, :, 3], op=add
            )

            # gathered reflection vector pairs (already scaled by sqrt(2))
            v01 = bigpool.tile([128, G, 2 * R], f16)
            v23 = bigpool.tile([128, G, 2 * R], f16)
            for g in range(G):
                nc.gpsimd.indirect_dma_start(
                    out=v01[:, g, :],
                    out_offset=None,
                    in_=pp_d[:],
                    in_offset=bass.IndirectOffsetOnAxis(ap=idx01[:, g:g + 1], axis=0),
                )
                nc.gpsimd.indirect_dma_start(
                    out=v23[:, g, :],
                    out_offset=None,
                    in_=pp_d[:],
                    in_offset=bass.IndirectOffsetOnAxis(ap=idx23[:, g:g + 1], axis=0),
                )

            y_all = bigpool.tile([128, G, R], f16)
            hh_scr = bigpool.tile([128, G, R], f16)
            md = bigpool.tile([128, G], f16)
            mdb = bigpool.tile([128, 5, R], f16)
            v_views = [
                v01[:, :, 0:R],
                v01[:, :, R:2 * R],
                v23[:, :, 0:R],
                v23[:, :, R:2 * R],
            ]

            def hh_chunk(start, ln, tail=False):
                s = slice(start, start + ln)
                for k in range(K):
                    vk = v_views[k][:, s, :]
                    nc.vector.tensor_tensor(
                        out=hh_scr[:, s, :], in0=y_all[:, s, :], in1=vk, op=mult
                    )
                    with nc.allow_low_precision("fp16 Householder dots"):
                        nc.vector.tensor_reduce(
                            out=md[:, s], in_=hh_scr[:, s, :], axis=X, op=add
                        )
                    if tail:
                        # stay on DVE: cross-engine hops cost serial latency
                        nc.vector.tensor_tensor(
                            out=hh_scr[:, s, :],
                            in0=vk,
                            in1=md[:, s].unsqueeze(2).broadcast_to([128, ln, R]),
                            op=mult,
                        )
                    else:
                        # materialize the broadcast on ScalarE so the DVE
                        # multiply keeps its unit-stride 2x mode
                        nc.scalar.copy(
                            mdb[:, 0:ln, :],
                            md[:, s].unsqueeze(2).broadcast_to([128, ln, R]),
                        )
                        nc.vector.tensor_tensor(
                            out=hh_scr[:, s, :], in0=vk, in1=mdb[:, 0:ln, :], op=mult
                        )
                    nc.vector.tensor_tensor(
                        out=y_all[:, s, :],
                        in0=y_all[:, s, :],
                        in1=hh_scr[:, s, :],
                        op=sub,
                    )
                y32 = sclpool.tile([128, ln, R], f32, tag="y32")
                nc.vector.tensor_copy(y32[:], y_all[:, s, :])
                nc.sync.dma_start(out_re[:, s, :], y32[:])

            # ---- projection + weighted sum (transposes run one tile
            # ahead so the PSUM->SBUF evacuation hides under matmuls) ----
            def stage_x(g):
                x_g = xpool.tile([128, D], f32r, tag="x")
                nc.sync.dma_start(x_g[:], x_re[:, g, :])
                xt_g = xtpool.tile([128, KC, 128], f16, tag="xt")
                for half in range(2):
                    ps_xt = psA.tile([128, KC // 2, 128], f32r, tag="psxt")
                    for cc in range(KC // 2):
                        c = half * (KC // 2) + cc
                        nc.tensor.transpose(
                            ps_xt[:, cc, :], x_g[:, c * 128:(c + 1) * 128], idm[:]
                        )
                    nc.scalar.copy(
                        xt_g[:, half * (KC // 2):(half + 1) * (KC // 2), :],
                        ps_xt[:],
                    )
                return xt_g

            xt_next = stage_x(0)
            for g in range(G):
                xt_g = xt_next
                if g + 1 < G:
                    xt_next = stage_x(g + 1)

                ps_proj = psB.tile([128, R, N_IN], f32, tag="psproj")
                for c in range(KC):
                    for h in range(2):
                        nc.tensor.matmul(
                            ps_proj[:, 64 * h:64 * h + 64, :],
                            lhsT=xt_g[:, c, :],
                            rhs=wf_sb[:, c, h * 512:(h + 1) * 512],
                            start=(c == 0),
                            stop=(c == KC - 1),
                        )

                scaled = sclpool.tile([128, R, N_IN], f16, tag="scl")
                with nc.allow_low_precision("fp16 weighted-sum tree"):
                    nc.vector.tensor_tensor(
                        out=scaled[:],
                        in0=ps_proj[:],
                        in1=iw_sb[:, g].unsqueeze(1).broadcast_to([128, R, N_IN]),
                        op=mult,
                    )
                    t1 = sclpool.tile([128, R, 4], f16, tag="t1")
                    nc.vector.tensor_tensor(
                        out=t1[:], in0=scaled[:, :, 0:4], in1=scaled[:, :, 4:8],
                        op=add,
                    )
                    t2 = sclpool.tile([128, R, 2], f16, tag="t2")
                    nc.vector.tensor_tensor(
                        out=t2[:], in0=t1[:, :, 0:2], in1=t1[:, :, 2:4], op=add
                    )
                    nc.vector.tensor_tensor(
                        out=y_all[:, g, :], in0=t2[:, :, 0], in1=t2[:, :, 1], op=add
                    )

                for cs, cl in HH_CHUNKS[:-1]:
                    if g == cs + cl - 1:
                        hh_chunk(cs, cl)

            hh_chunk(*HH_CHUNKS[-1], tail=True)

    nc.compile()
    return nc


def _get_program():
    if "nc" not in _cache:
        _cache["nc"] = _build_program()
    return _cache["nc"]


def _host_prep(x, input_weights, process_indices, input_neurons, process_neurons):
    xf = np.ascontiguousarray(np.asarray(x, dtype=np.float32)).reshape(T_TOTAL, D)
    iwf = np.ascontiguousarray(np.asarray(input_weights, dtype=np.float32)).reshape(
        T_TOTAL, N_IN
    )
    idxf = np.ascontiguousarray(np.asarray(process_indices, dtype=np.int32)).reshape(
        T_TOTAL, K
    )
    # W layout: wflat[d, r*8+n] = input_neurons[n, d, r]  (r-major, n innermost)
    wflat = np.ascontiguousarray(
        np.transpose(np.asarray(input_neurons, dtype=np.float32), (1, 2, 0)).reshape(
            D, R * N_IN
        )
    ).astype(np.float16)
    ident = np.eye(128, dtype=np.float32)
    # pair table: row j0*32+j1 = sqrt(2) * [P[j0] | P[j1]]
    # (sqrt(2) scaling turns y - (v*sqrt2)((v*sqrt2).y) into y - 2 v (v.y))
    pn = np.asarray(process_neurons, dtype=np.float32) * np.float32(np.sqrt(2.0))
    ppair = np.concatenate(
        [
            np.repeat(pn, N_PROC, axis=0),
            np.tile(pn, (N_PROC, 1)),
        ],
        axis=1,
    ).astype(np.float16)
    in_maps = []
    for c in range(N_CORES):
        sl = slice(c * T, (c + 1) * T)
        in_maps.append(
            {
                "x": xf[sl],
                "iw": iwf[sl],
                "pidx": idxf[sl],
                "wflat": wflat,
                "ident": ident,
                "ppair": ppair,
            }
        )
    return in_maps


def kernel(x, input_weights, process_indices, input_neurons, process_neurons):
    global last_results
    from concourse.bass_utils import run_bass_kernel_spmd

    nc = _get_program()
    in_maps = _host_prep(
        x, input_weights, process_indices, input_neurons, process_neurons
    )
    res = run_bass_kernel_spmd(nc, in_maps, core_ids=list(range(N_CORES)))
    last_results = res
    out = np.concatenate([res.results[c]["out"] for c in range(N_CORES)], axis=0)
    return out.reshape(B, S, R)
